# revision 32
# baseline (speedup 1.0000x reference)
"""CIEDE2000 ColorLoss kernel v2.1 for Trainium2, 8 NeuronCores, data-parallel.

Full inputs x, y: [32, 3, 512, 512] f32 NCHW in [0, 1).
Output: scalar f32 = mean(ciede2000(rgb2lab(x), rgb2lab(y))) / 100.

v2 design (vs the v1 baseline):
  - ONE ACT table set (natural_log_exp): all roots/powers as Ln/Exp pairs,
    no Arctan/Sin tables -> no ACT_TABLE_LOAD thrash.
  - No GPSIMD tensor_scalar (19.7us each on HW); GPSIMD only runs plain
    tensor_tensor add/sub/mult offloads.
  - Hue handled vectorially (no angles): (cos h, sin h) unit vectors, the
    CIEDE2000 hbar wrap rule == bisector of the smaller arc, so
    (cos hbar, sin hbar) = normalize(u1+u2). T expands exactly as
    P4(cos hbar) + sin hbar * Q3(cos hbar).
  - dHp via dHp^2 = 2*(c1'c2' - 25a1'a2' - 4b1b2), sign from the cross
    product; dtheta gaussian via cos(hbar-275deg) small-angle identity.
  - sRGB linearization: single fused deg-3 poly (custom DVE op), dark-branch
    dropped (validated: end-to-end rel err ~1e-4).
  - G / RC chroma sigmoids: s^7 power chain + approx reciprocal + deg-3 poly
    of sqrt(1-z), all on DVE.
  - bf16 storage for most intermediates (DVE 2x modes); fp32 where bit
    tricks (reciprocal seed) or accuracy demand it.
  - 12 new fused custom DVE ops registered at import (runtime extension of
    concourse.dve_ops.OPS).
"""
import os
import re
import sys

sys.path.insert(0, "/opt/trn_rl_repo")

import numpy as np
import concourse.bacc as bacc
import concourse.tile as tile
import concourse.mybir as mybir
from concourse.bass_utils import run_bass_kernel_spmd
from contextlib import ExitStack

F32 = mybir.dt.float32
BF16 = mybir.dt.bfloat16
AF = mybir.ActivationFunctionType
ALU = mybir.AluOpType

P = 128
S = 1024          # chunk free dim (pixel pairs per partition-col block)
D = 2 * S         # both-images width
M = 3 * D         # 3 channels / comps width
NCHUNK = 8
NCORE = 8
IMGS_PER_CORE = 4
ROWS_PER_IMG = 32

# ---------------- constants ----------------
_M = np.array([[0.412453, 0.357580, 0.180423],
               [0.212671, 0.715160, 0.072169],
               [0.019334, 0.119193, 0.950227]], dtype=np.float64)
_W = np.array([0.95047, 1.0, 1.08883], dtype=np.float64)
MW = (_M / _W[:, None])            # [3,3] row k = xyz_k coeffs

_d = np.deg2rad
COS30, SIN30 = np.cos(_d(30)), np.sin(_d(30))
COS6, SIN6 = np.cos(_d(6)), np.sin(_d(6))
COS63, SIN63 = np.cos(_d(63)), np.sin(_d(63))
COS275, SIN275 = np.cos(_d(275)), np.sin(_d(275))

# T = P4(cb) + sb*Q3(cb)  (exact 4-harmonic expansion)
E4 = -1.6 * COS63
E3 = 1.28 * COS6
E2 = 0.48 + 1.6 * COS63
E1 = -0.17 * COS30 - 0.96 * COS6
E0 = 1.0 - 0.24 - 0.20 * COS63
Q3_ = [-1.6 * SIN63, -1.28 * SIN6, 0.8 * SIN63, 0.32 * SIN6 - 0.085]

K1 = 1.0471976          # 60deg in rad (2*dtheta = K1 * e)
K3 = -K1 ** 3 / 6.0
K5 = K1 ** 5 / 120.0
KG = 2.0 * (180.0 / np.pi) ** 2 / 625.0   # 10.50499: (hbar-275)^2/25^2 ~ KG*(1-cos)

PWA, PWB = 0.9398, 0.3893   # 1-seg PWL hypot coefficients


def _fit_poly(f, lo, hi, deg, w=None, n=20001):
    x = np.linspace(lo, hi, n)
    yv = f(x)
    Wt = np.ones_like(x) if w is None else w(x)
    V = np.vander(x, deg + 1)
    c, *_ = np.linalg.lstsq(V * Wt[:, None], yv * Wt, rcond=None)
    return c            # highest power first


LINC = _fit_poly(lambda x: ((x + 0.055) / 1.055) ** 2.4, 0.0, 1.0, 3)
SQ1Z = _fit_poly(lambda z: np.sqrt(1.0 - z), 0.0, 1.0, 3,
                 w=lambda z: 1.0 / np.sqrt(0.005 + z))
GC = -0.5 * SQ1Z                   # g1 = 1.5 - 0.5*p(z)
GC[-1] += 1.5

_NC_CACHE = {}

# Force every ACT op onto the combined ln+exp table set: hide all other
# sets from the table-load chooser (indices preserved; set 6 has ln, exp,
# square, sign, relu, abs).
import concourse.bacc as _bacc_mod
from concourse.hw_specs import get_activation_tables as _gat_orig

def _gat_one_set(arch):
    tabs = _gat_orig(arch)
    keep = "natural_log_exp_and_others"
    return {n: (f if n == keep else set()) for n, f in tabs.items()}

_bacc_mod.get_activation_tables = _gat_one_set

# ---------------- custom DVE ops ----------------
_OPS = {}


def _register_custom_ops():
    if _OPS:
        return _OPS
    import concourse.dve_ops as _dm
    prior = {o.name: o for o in _dm.OPS if o.name.startswith("ANT_")}
    if prior:            # already registered in this process (re-import)
        _OPS.update(prior)
        return _OPS
    from concourse.dve_spec import (
        Spec, Src0, Src1, C0, C1, C2, C3, Zero, One, relu, maxx, minn, lower,
        _spill_c3_to_src1,
    )
    import concourse.dve_ops as dmod

    def mk(name, body, ref, spill=False):
        spec = Spec(body=_spill_c3_to_src1(body) if spill else body,
                    reference=ref)
        row = max(dmod._SUB_OPCODE_FOR_NAME.values()) + 1
        assert row < 0x20, "custom DVE opcode rows exhausted"
        dmod._SUB_OPCODE_FOR_NAME[name] = row
        op = dmod.DveOp(name, spec, subdim=False, uops_sha={})
        for ver in ("v3", "v4"):
            try:
                op.compile(ver)
            except ValueError as e:
                m = re.search(r"([0-9a-f]{16})\s*≠\s*pinned", str(e))
                if not m:
                    raise
                op.uops_sha[ver] = m.group(1)
                op.compile(ver)
            except Exception:
                pass        # ver not supported; fine if unused
        dmod.OPS.append(op)
        dmod.CUSTOM_DVE_SPECS[name] = op.spec
        _OPS[name] = op
        return op

    # deg-3 Horner; d0 spilled to Src1 ([P,1] broadcast tile)
    mk("ANT_POLY3", ((C0 * Src0 + C1) * Src0 + C2) * Src0 + C3,
       lambda in0, in1, s0, s1, imm2:
           (((s0 * in0.astype(np.float32) + s1) * in0 + imm2) * in0
            + in1).astype(np.float32), spill=True)
    # deg-2 Horner
    mk("ANT_POLY2", (C0 * Src0 + C1) * Src0 + C2,
       lambda in0, in1, s0, s1, imm2:
           ((s0 * in0.astype(np.float32) + s1) * in0 + imm2).astype(np.float32))
    # p = (h*c + C0)*c + C1   (tail of deg-4 Horner; in0=c, in1=h)
    mk("ANT_TP2B", (Src1 * Src0 + C0) * Src0 + C1,
       lambda in0, in1, s0, s1, imm2:
           ((in1.astype(np.float32) * in0 + s0) * in0 + s1).astype(np.float32))
    # (C0*a)^2 + (C1*b)^2
    _qa = (Src0 * C0) * (Src0 * C0)
    _qb = (Src1 * C1) * (Src1 * C1)
    mk("ANT_Q2", _qa + _qb,
       lambda in0, in1, s0, s1, imm2:
           ((s0 * in0.astype(np.float32)) ** 2 + (s1 * in1) ** 2)
           .astype(np.float32))
    # C0 * (a+b)^7 + C1  (fused chroma-sum sigmoid input)
    _ss = Src0 + Src1
    _t2 = _ss * _ss
    _t4 = _t2 * _t2
    mk("ANT_S7C2", (_t4 * _t2 * _ss) * C0 + C1,
       lambda in0, in1, s0, s1, imm2:
           (s0 * (in0.astype(np.float64) + in1) ** 7 + s1).astype(np.float32))
    # (a*b)^2
    _ab = Src0 * Src1
    mk("ANT_TSQ", _ab * _ab,
       lambda in0, in1, s0, s1, imm2:
           ((in0.astype(np.float32) * in1) ** 2).astype(np.float32))
    # C0 * s^7 + C1
    _s2 = Src0 * Src0
    _s4 = _s2 * _s2
    _s7 = _s4 * _s2 * Src0
    mk("ANT_S7C", _s7 * C0 + C1,
       lambda in0, in1, s0, s1, imm2:
           (s0 * in0.astype(np.float64) ** 7 + s1).astype(np.float32))
    # C0*a + C1*b
    mk("ANT_DOT2", Src0 * C0 + Src1 * C1,
       lambda in0, in1, s0, s1, imm2:
           (s0 * in0.astype(np.float32) + s1 * in1).astype(np.float32))
    # e * (K1 + K3 e^2 + K5 e^4): sin(K1*e) odd poly
    _e2 = Src0 * Src0
    mk("ANT_SINP", ((C0 * _e2 + C1) * _e2 + C2) * Src0,
       lambda in0, in1, s0, s1, imm2:
           (((s0 * in0.astype(np.float32) ** 2 + s1) * in0 ** 2 + imm2) * in0)
           .astype(np.float32))
    # a^2 + b^2
    mk("ANT_FQ1", Src0 * Src0 + Src1 * Src1,
       lambda in0, in1, s0, s1, imm2:
           (in0.astype(np.float32) ** 2 + in1.astype(np.float32) ** 2)
           .astype(np.float32))
    # a^2 + b
    mk("ANT_FQ2", Src0 * Src0 + Src1,
       lambda in0, in1, s0, s1, imm2:
           (in0.astype(np.float32) ** 2 + in1).astype(np.float32))
    # relu(a + b)
    mk("ANT_FSR", relu(Src0 + Src1),
       lambda in0, in1, s0, s1, imm2:
           np.maximum(in0.astype(np.float32) + in1, 0.0).astype(np.float32))
    # ((a*C0)*b + 1) * C1   (S-scale with folded output scale)
    mk("ANT_AFFS", ((Src0 * C0) * Src1 + One) * C1,
       lambda in0, in1, s0, s1, imm2:
           ((1.0 + s0 * in0.astype(np.float32) * in1) * s1).astype(np.float32))
    return _OPS


_BIASES = [0.055 / 1.055, -66.0, 20.0, -float(KG), 1.0, float(np.log(2.0))]


def _reg_consts(nc, values):
    for v in values:
        v = float(v)
        if (F32, v) not in nc.const_aps.aps:
            t = nc.alloc_sbuf_tensor(f"constf32_{repr(v)}", [128, 1], F32)
            nc.gpsimd.memset(t.ap(), v)
            nc.const_aps.aps[(F32, v)] = t.ap()
    nc.all_engine_barrier()


# tt ops that may be offloaded to GPSIMD (plain add/sub/mult only)
GP_OPS = {"m1", "m2", "m3", "v1", "v2", "tu", "Tt", "Wa", "Wb", "sfy", "ct",
          "dfy", "dCp", "cbb", "sbb", "rr"}


def build_nc(use_gp=True):
    ops = _register_custom_ops()
    POLY3, POLY2, TP2B = ops["ANT_POLY3"], ops["ANT_POLY2"], ops["ANT_TP2B"]
    Q2, S7C = ops["ANT_Q2"], ops["ANT_S7C"]
    DOT2, SINP = ops["ANT_DOT2"], ops["ANT_SINP"]
    FQ1, FQ2, FSR = ops["ANT_FQ1"], ops["ANT_FQ2"], ops["ANT_FSR"]
    AFFS = ops["ANT_AFFS"]
    S7C2, TSQ = ops["ANT_S7C2"], ops["ANT_TSQ"]

    nc = bacc.Bacc("TRN2", target_bir_lowering=False, debug=False)
    _reg_consts(nc, _BIASES)
    A = nc.scalar
    V = nc.vector
    Gp = nc.gpsimd

    shp = [IMGS_PER_CORE, 3, ROWS_PER_IMG, NCHUNK, S]
    x_d = nc.dram_tensor("x", shp, F32, kind="ExternalInput").ap()
    y_d = nc.dram_tensor("y", shp, F32, kind="ExternalInput").ap()
    out_d = nc.dram_tensor("out", [P, 1], F32, kind="ExternalOutput").ap()

    with tile.TileContext(nc) as tc, ExitStack() as ctx:
        pool = ctx.enter_context(tc.tile_pool(name="main", bufs=1))

        def TL(tag, w, dt):
            return pool.tile([P, w], dt, tag=tag, name=tag)

        # broadcast-constant [P,1] tiles (POLY3 d0 via C3-spill)
        def bctile(tag, val):
            t = pool.tile([P, 1], F32, tag=tag, name=tag)
            nc.gpsimd.memset(t[:], float(val))
            return t

        bc_lin = bctile("bc_lin", LINC[3])
        bc_g = bctile("bc_g", GC[3])
        bc_rc = bctile("bc_rc", SQ1Z[3])
        bc_tq = bctile("bc_tq", Q3_[3])
        nc.all_engine_barrier()

        acc = pool.tile([P, NCHUNK], F32, tag="acc", name="acc")

        NTMPF = 3      # rotating [P,S] f32 scratch
        NTMPB = 7      # rotating [P,S] bf16 scratch
        tmpi = [0, 0]

        def tmpf():
            t = TL(f"tmpf{tmpi[0] % NTMPF}", S, F32)
            tmpi[0] += 1
            return t

        def tmpb():
            t = TL(f"tmpb{tmpi[1] % NTMPB}", S, BF16)
            tmpi[1] += 1
            return t

        def emit_tt(name, out, a, b, op):
            if use_gp and name in GP_OPS:
                Gp.tensor_tensor(out[:], a, b, op)
            else:
                V.tensor_tensor(out[:], a, b, op)

        for k in range(NCHUNK):
            # ---- DMA input: rgb mega tile [P, 6144] = 3ch x (x|y) ----
            # alternating tag: chunk k+1 DMA overlaps chunk k compute
            rgb = TL(f"mega_rgb{k % 2}", M, F32)
            for c in range(3):
                for im in range(IMGS_PER_CORE):
                    pr = slice(im * ROWS_PER_IMG, (im + 1) * ROWS_PER_IMG)
                    nc.sync.dma_start(rgb[pr, c * D:c * D + S],
                                      x_d[im, c, :, k, :])
                    nc.sync.dma_start(rgb[pr, c * D + S:(c + 1) * D],
                                      y_d[im, c, :, k, :])

            # ---- linearize: deg-3 poly per channel (finer overlap) ----
            lin = TL("mega_lin", M, BF16)
            for c in range(3):
                V._custom_dve(POLY3, out=lin[:, c * D:(c + 1) * D],
                              in0=rgb[:, c * D:(c + 1) * D], in1=bc_lin[:],
                              s0=float(LINC[0]), s1=float(LINC[1]),
                              imm2=float(LINC[2]))

            # ---- xyz mix (per comp: DOT2 + stt) ----
            X = TL("mega_x", M, BF16)
            for kk in range(3):
                lr = lin[:, 0:D]
                lg = lin[:, D:2 * D]
                lb = lin[:, 2 * D:3 * D]
                t0 = TL("dT0", D, BF16)
                V._custom_dve(DOT2, out=t0[:], in0=lr, in1=lg,
                              s0=float(MW[kk, 0]), s1=float(MW[kk, 1]))
                V.scalar_tensor_tensor(X[:, kk * D:(kk + 1) * D], lb,
                                       float(MW[kk, 2]), t0[:],
                                       ALU.mult, ALU.add)

            # ---- cbrt via Ln/Exp per comp (one table set) ----
            LX = TL(f"mega_rgb{k % 2}", M, F32)  # rgb dead after lin poly
            f = TL("mega_f", M, BF16)
            for c in range(3):
                sl = slice(c * D, (c + 1) * D)
                A.activation(LX[:, sl], X[:, sl], AF.Ln)
                A.activation(f[:, sl], LX[:, sl], AF.Exp,
                             scale=float(1.0 / 3.0))

            fx = f[:, 0:D]
            fy = f[:, D:2 * D]
            fz = f[:, 2 * D:3 * D]
            fy1 = f[:, D:D + S]
            fy2 = f[:, D + S:2 * D]

            alpha = TL("alpha", D, BF16)
            emit_tt("alpha", alpha, fx, fy, ALU.subtract)
            beta = TL("beta", D, BF16)
            emit_tt("beta", beta, fy, fz, ALU.subtract)
            be1 = beta[:, 0:S]
            be2 = beta[:, S:D]

            # ---- pre-G chroma (exact, ACT sqrt) + G sigmoid ----
            qpre = TL("dP", D, F32)
            V._custom_dve(Q2, out=qpre[:], in0=alpha[:], in1=beta[:],
                          s0=5.0, s1=2.0)
            A.activation(qpre[:], qpre[:], AF.Ln)
            cpw = TL("dW", D, BF16)
            A.activation(cpw[:], qpre[:], AF.Exp, scale=0.5)
            z7 = tmpf()
            V._custom_dve(S7C2, out=z7[:], in0=cpw[:, 0:S], in1=cpw[:, S:D],
                          s0=128.0, s1=1.0)
            lz7 = tmpf()
            A.activation(lz7[:], z7[:], AF.Ln)
            zr = tmpf()
            A.activation(zr[:], lz7[:], AF.Exp, scale=-1.0)
            lz = tmpf()
            A.activation(lz[:], zr[:], AF.Ln, scale=-1.0, bias=1.0)
            wg = tmpb()
            A.activation(wg[:], lz[:], AF.Exp, scale=0.5)
            g1 = TL("g1", S, BF16)
            V.tensor_scalar(g1[:], wg[:], -0.5, 1.5, ALU.mult, ALU.add)

            # ---- post-G chroma ----
            ap = TL("ap", D, BF16)
            V.tensor_tensor(ap[:, 0:S], alpha[:, 0:S], g1[:], ALU.mult)
            V.tensor_tensor(ap[:, S:D], alpha[:, S:D], g1[:], ALU.mult)
            qp = TL("dQ", D, F32)
            V._custom_dve(Q2, out=qp[:], in0=ap[:], in1=beta[:],
                          s0=5.0, s1=2.0)
            A.activation(qp[:], qp[:], AF.Ln)
            yq = TL("yq", D, BF16)
            A.activation(yq[:], qp[:], AF.Exp, scale=-0.5)
            cp = TL("cp", D, BF16)
            A.activation(cp[:], qp[:], AF.Exp, scale=0.5)

            dCp = TL("dCp", S, BF16)
            emit_tt("dCp", dCp, cp[:, S:D], cp[:, 0:S], ALU.subtract)
            Scp = TL("Scp", S, F32)
            V.tensor_tensor(Scp[:], cp[:, 0:S], cp[:, S:D], ALU.add)
            SCf = tmpf()
            V.tensor_scalar(SCf[:], Scp[:], 0.0225, 0.01, ALU.mult, ALU.add)
            lsc = tmpf()
            A.activation(lsc[:], SCf[:], AF.Ln)
            iC = TL("iC", S, F32)
            A.activation(iC[:], lsc[:], AF.Exp, scale=-1.0)

            # ---- RC sigmoid (same shape as G) ----
            z7c = tmpf()
            V._custom_dve(S7C, out=z7c[:], in0=Scp[:], s0=128.0, s1=1.0)
            lz7c = tmpf()
            A.activation(lz7c[:], z7c[:], AF.Ln)
            zrc = tmpf()
            A.activation(zrc[:], lz7c[:], AF.Exp, scale=-1.0)
            lzc = tmpf()
            A.activation(lzc[:], zrc[:], AF.Ln, scale=-1.0, bias=1.0)
            rsqC = TL("rsqC", S, BF16)
            A.activation(rsqC[:], lzc[:], AF.Exp, scale=0.5)

            # ---- SL ----
            sfy = TL("sfy", S, BF16)
            emit_tt("sfy", sfy, fy1, fy2, ALU.add)
            dfy = TL("dfy", S, BF16)
            emit_tt("dfy", dfy, fy2, fy1, ALU.subtract)
            L50 = TL("L50", S, F32)
            A.activation(L50[:], sfy[:], AF.Square, scale=58.0, bias=-66.0)
            l20 = tmpf()
            A.activation(l20[:], L50[:], AF.Ln, bias=20.0)
            rsq20 = tmpb()
            A.activation(rsq20[:], l20[:], AF.Exp, scale=-0.5)
            SLf = tmpf()
            V._custom_dve(AFFS, out=SLf[:], in0=L50[:], in1=rsq20[:],
                          s0=0.015, s1=float(1.0 / 116.0))
            iL = TL("iL", S, F32)
            V.reciprocal_approx_fast(iL[:], SLf[:])

            # ---- unit chroma vectors + bisector ----
            ca = TL("dA", D, BF16)
            V.scalar_tensor_tensor(ca[:], ap[:], 5.0, yq[:], ALU.mult,
                                   ALU.mult)
            cbt = TL("dB", D, BF16)
            V.scalar_tensor_tensor(cbt[:], beta[:], 2.0, yq[:], ALU.mult,
                                   ALU.mult)
            Wa = TL("Wa", S, BF16)
            emit_tt("Wa", Wa, ca[:, 0:S], ca[:, S:D], ALU.add)
            Wb = TL("Wb", S, BF16)
            emit_tt("Wb", Wb, cbt[:, 0:S], cbt[:, S:D], ALU.add)
            n2 = tmpf()
            V._custom_dve(Q2, out=n2[:], in0=Wa[:], in1=Wb[:], s0=1.0, s1=1.0)
            ln2 = tmpf()
            A.activation(ln2[:], n2[:], AF.Ln)
            rn = tmpb()
            A.activation(rn[:], ln2[:], AF.Exp, scale=-0.5)
            cbb = TL("cbb", S, BF16)
            emit_tt("cbb", cbb, Wa[:], rn[:], ALU.mult)
            sbb = TL("sbb", S, BF16)
            emit_tt("sbb", sbb, Wb[:], rn[:], ALU.mult)

            # ---- T = P4(cb) + sb*Q3(cb) ----
            q3t = tmpb()
            V._custom_dve(POLY3, out=q3t[:], in0=cbb[:], in1=bc_tq[:],
                          s0=float(Q3_[0]), s1=float(Q3_[1]),
                          imm2=float(Q3_[2]))
            hh = tmpb()
            V._custom_dve(POLY2, out=hh[:], in0=cbb[:],
                          s0=float(E4), s1=float(E3), imm2=float(E2))
            pp = tmpb()
            V._custom_dve(TP2B, out=pp[:], in0=cbb[:], in1=hh[:],
                          s0=float(E1), s1=float(E0))
            tu = tmpb()
            emit_tt("tu", tu, sbb[:], q3t[:], ALU.mult)
            Tt = tmpb()
            emit_tt("Tt", Tt, pp[:], tu[:], ALU.add)
            SHf = tmpf()
            V._custom_dve(AFFS, out=SHf[:], in0=Scp[:], in1=Tt[:],
                          s0=0.75, s1=0.01)
            lsh = tmpf()
            A.activation(lsh[:], SHf[:], AF.Ln)
            iH = TL("iH", S, F32)
            A.activation(iH[:], lsh[:], AF.Exp, scale=-1.0)

            # ---- RT: gaussian via cos identity + sin poly ----
            cd = tmpb()
            V._custom_dve(DOT2, out=cd[:], in0=cbb[:], in1=sbb[:],
                          s0=float(COS275), s1=float(SIN275))
            ee = tmpb()
            A.activation(ee[:], cd[:], AF.Exp, scale=float(KG),
                         bias=-float(KG))
            s2d = tmpb()
            V._custom_dve(SINP, out=s2d[:], in0=ee[:],
                          s0=float(K5), s1=float(K3), imm2=float(K1))
            rr = TL("rr", S, BF16)
            emit_tt("rr", rr, s2d[:], rsqC[:], ALU.mult)

            # ---- signed dHp = 2*sqrt(c1'c2') * sin(dh/2) ----
            # sin(dh/2) = (u1 x u2) * rn exactly (sign automatic)
            lqs = tmpf()
            V.tensor_tensor(lqs[:], qp[:, 0:S], qp[:, S:D], ALU.add)
            sq2 = tmpb()
            A.activation(sq2[:], lqs[:], AF.Exp, scale=0.25,
                         bias=float(np.log(2.0)))
            v1 = tmpb()
            emit_tt("v1", v1, ca[:, 0:S], cbt[:, S:D], ALU.mult)
            v2 = tmpb()
            emit_tt("v2", v2, cbt[:, 0:S], ca[:, S:D], ALU.mult)
            cr = tmpf()
            V.tensor_tensor(cr[:], v1[:], v2[:], ALU.subtract)
            sdh = tmpb()
            V.tensor_tensor(sdh[:], cr[:], rn[:], ALU.mult)
            hn = tmpb()
            V.tensor_tensor(hn[:], sdh[:], sq2[:], ALU.mult)

            # ---- assemble F ----
            tL2 = tmpb()
            V._custom_dve(TSQ, out=tL2[:], in0=dfy[:], in1=iL[:])
            tC = tmpb()
            emit_tt("tC", tC, dCp[:], iC[:], ALU.mult)
            tH = tmpb()
            emit_tt("tH", tH, hn[:], iH[:], ALU.mult)
            F1 = tmpb()
            V._custom_dve(FQ2, out=F1[:], in0=tC[:], in1=tL2[:])
            F2 = tmpb()
            V._custom_dve(FQ2, out=F2[:], in0=tH[:], in1=F1[:])
            ct = tmpb()
            emit_tt("ct", ct, tC[:], tH[:], ALU.mult)
            ctr = tmpb()
            V.scalar_tensor_tensor(ctr[:], rr[:], -2.0, ct[:], ALU.mult,
                                   ALU.mult)
            FS = tmpf()
            V._custom_dve(FSR, out=FS[:], in0=F2[:], in1=ctr[:])
            lF = tmpf()
            A.activation(lF[:], FS[:], AF.Ln)
            dE = tmpf()
            A.activation(dE[:], lF[:], AF.Exp, scale=0.5,
                         accum_out=acc[:, k:k + 1])

        accsum = pool.tile([P, 1], F32, tag="accsum", name="accsum")
        V.tensor_reduce(accsum[:], acc[:], mybir.AxisListType.X, ALU.add)
        nc.sync.dma_start(out_d[:], accsum[:])

    nc.compile()
    return nc


def _get_nc():
    if "nc" not in _NC_CACHE:
        _NC_CACHE["nc"] = build_nc(use_gp=bool(int(
            os.environ.get("COLOR_GP", "1"))))
    return _NC_CACHE["nc"]


def kernel(x: np.ndarray, y: np.ndarray) -> np.ndarray:
    assert x.shape == (32, 3, 512, 512) and y.shape == (32, 3, 512, 512)
    nc = _get_nc()
    shp = (IMGS_PER_CORE, 3, ROWS_PER_IMG, NCHUNK, S)
    xs = np.ascontiguousarray(x, dtype=np.float32)
    ys = np.ascontiguousarray(y, dtype=np.float32)
    in_maps = []
    for c in range(NCORE):
        xi = xs[c * IMGS_PER_CORE:(c + 1) * IMGS_PER_CORE].reshape(shp)
        yi = ys[c * IMGS_PER_CORE:(c + 1) * IMGS_PER_CORE].reshape(shp)
        in_maps.append({"x": xi, "y": yi})
    trace = bool(int(os.environ.get("COLOR_TRACE", "0")))
    res = run_bass_kernel_spmd(nc, in_maps, core_ids=list(range(NCORE)),
                               trace=trace)
    _NC_CACHE["last_results"] = res
    total = np.float64(0.0)
    for c in range(NCORE):
        total += np.float64(res.results[c]["out"].sum())
    npix = 32 * 512 * 512
    return np.float32(total / npix / 100.0)


# revision 33
# speedup vs baseline: 1.0118x; 1.0118x over previous
"""CIEDE2000 ColorLoss kernel v2.1 for Trainium2, 8 NeuronCores, data-parallel.

Full inputs x, y: [32, 3, 512, 512] f32 NCHW in [0, 1).
Output: scalar f32 = mean(ciede2000(rgb2lab(x), rgb2lab(y))) / 100.

v2 design (vs the v1 baseline):
  - ONE ACT table set (natural_log_exp): all roots/powers as Ln/Exp pairs,
    no Arctan/Sin tables -> no ACT_TABLE_LOAD thrash.
  - No GPSIMD tensor_scalar (19.7us each on HW); GPSIMD only runs plain
    tensor_tensor add/sub/mult offloads.
  - Hue handled vectorially (no angles): (cos h, sin h) unit vectors, the
    CIEDE2000 hbar wrap rule == bisector of the smaller arc, so
    (cos hbar, sin hbar) = normalize(u1+u2). T expands exactly as
    P4(cos hbar) + sin hbar * Q3(cos hbar).
  - dHp via dHp^2 = 2*(c1'c2' - 25a1'a2' - 4b1b2), sign from the cross
    product; dtheta gaussian via cos(hbar-275deg) small-angle identity.
  - sRGB linearization: single fused deg-3 poly (custom DVE op), dark-branch
    dropped (validated: end-to-end rel err ~1e-4).
  - G / RC chroma sigmoids: s^7 power chain + approx reciprocal + deg-3 poly
    of sqrt(1-z), all on DVE.
  - bf16 storage for most intermediates (DVE 2x modes); fp32 where bit
    tricks (reciprocal seed) or accuracy demand it.
  - 12 new fused custom DVE ops registered at import (runtime extension of
    concourse.dve_ops.OPS).
"""
import os
import re
import sys

sys.path.insert(0, "/opt/trn_rl_repo")

import numpy as np
import concourse.bacc as bacc
import concourse.tile as tile
import concourse.mybir as mybir
from concourse.bass_utils import run_bass_kernel_spmd
from contextlib import ExitStack

F32 = mybir.dt.float32
BF16 = mybir.dt.bfloat16
AF = mybir.ActivationFunctionType
ALU = mybir.AluOpType

P = 128
S = 1024          # chunk free dim (pixel pairs per partition-col block)
D = 2 * S         # both-images width
M = 3 * D         # 3 channels / comps width
NCHUNK = 8
NCORE = 8
IMGS_PER_CORE = 4
ROWS_PER_IMG = 32

# ---------------- constants ----------------
_M = np.array([[0.412453, 0.357580, 0.180423],
               [0.212671, 0.715160, 0.072169],
               [0.019334, 0.119193, 0.950227]], dtype=np.float64)
_W = np.array([0.95047, 1.0, 1.08883], dtype=np.float64)
MW = (_M / _W[:, None])            # [3,3] row k = xyz_k coeffs

_d = np.deg2rad
COS30, SIN30 = np.cos(_d(30)), np.sin(_d(30))
COS6, SIN6 = np.cos(_d(6)), np.sin(_d(6))
COS63, SIN63 = np.cos(_d(63)), np.sin(_d(63))
COS275, SIN275 = np.cos(_d(275)), np.sin(_d(275))

# T = P4(cb) + sb*Q3(cb)  (exact 4-harmonic expansion)
E4 = -1.6 * COS63
E3 = 1.28 * COS6
E2 = 0.48 + 1.6 * COS63
E1 = -0.17 * COS30 - 0.96 * COS6
E0 = 1.0 - 0.24 - 0.20 * COS63
Q3_ = [-1.6 * SIN63, -1.28 * SIN6, 0.8 * SIN63, 0.32 * SIN6 - 0.085]

K1 = 1.0471976          # 60deg in rad (2*dtheta = K1 * e)
K3 = -K1 ** 3 / 6.0
K5 = K1 ** 5 / 120.0
KG = 2.0 * (180.0 / np.pi) ** 2 / 625.0   # 10.50499: (hbar-275)^2/25^2 ~ KG*(1-cos)

PWA, PWB = 0.9398, 0.3893   # 1-seg PWL hypot coefficients


def _fit_poly(f, lo, hi, deg, w=None, n=20001):
    x = np.linspace(lo, hi, n)
    yv = f(x)
    Wt = np.ones_like(x) if w is None else w(x)
    V = np.vander(x, deg + 1)
    c, *_ = np.linalg.lstsq(V * Wt[:, None], yv * Wt, rcond=None)
    return c            # highest power first


LINC = _fit_poly(lambda x: ((x + 0.055) / 1.055) ** 2.4, 0.0, 1.0, 3)
SQ1Z = _fit_poly(lambda z: np.sqrt(1.0 - z), 0.0, 1.0, 3,
                 w=lambda z: 1.0 / np.sqrt(0.005 + z))
GC = -0.5 * SQ1Z                   # g1 = 1.5 - 0.5*p(z)
GC[-1] += 1.5

_NC_CACHE = {}

# Force every ACT op onto the combined ln+exp table set: hide all other
# sets from the table-load chooser (indices preserved; set 6 has ln, exp,
# square, sign, relu, abs).
import concourse.bacc as _bacc_mod
from concourse.hw_specs import get_activation_tables as _gat_orig

def _gat_one_set(arch):
    tabs = _gat_orig(arch)
    keep = "natural_log_exp_and_others"
    return {n: (f if n == keep else set()) for n, f in tabs.items()}

_bacc_mod.get_activation_tables = _gat_one_set

# ---------------- custom DVE ops ----------------
_OPS = {}


def _register_custom_ops():
    if _OPS:
        return _OPS
    import concourse.dve_ops as _dm
    prior = {o.name: o for o in _dm.OPS if o.name.startswith("ANT_")}
    if prior:            # already registered in this process (re-import)
        _OPS.update(prior)
        return _OPS
    from concourse.dve_spec import (
        Spec, Src0, Src1, C0, C1, C2, C3, Zero, One, relu, maxx, minn, lower,
        _spill_c3_to_src1,
    )
    import concourse.dve_ops as dmod

    def mk(name, body, ref, spill=False):
        spec = Spec(body=_spill_c3_to_src1(body) if spill else body,
                    reference=ref)
        row = max(dmod._SUB_OPCODE_FOR_NAME.values()) + 1
        assert row < 0x20, "custom DVE opcode rows exhausted"
        dmod._SUB_OPCODE_FOR_NAME[name] = row
        op = dmod.DveOp(name, spec, subdim=False, uops_sha={})
        for ver in ("v3", "v4"):
            try:
                op.compile(ver)
            except ValueError as e:
                m = re.search(r"([0-9a-f]{16})\s*≠\s*pinned", str(e))
                if not m:
                    raise
                op.uops_sha[ver] = m.group(1)
                op.compile(ver)
            except Exception:
                pass        # ver not supported; fine if unused
        dmod.OPS.append(op)
        dmod.CUSTOM_DVE_SPECS[name] = op.spec
        _OPS[name] = op
        return op

    # deg-3 Horner; d0 spilled to Src1 ([P,1] broadcast tile)
    mk("ANT_POLY3", ((C0 * Src0 + C1) * Src0 + C2) * Src0 + C3,
       lambda in0, in1, s0, s1, imm2:
           (((s0 * in0.astype(np.float32) + s1) * in0 + imm2) * in0
            + in1).astype(np.float32), spill=True)
    # deg-2 Horner
    mk("ANT_POLY2", (C0 * Src0 + C1) * Src0 + C2,
       lambda in0, in1, s0, s1, imm2:
           ((s0 * in0.astype(np.float32) + s1) * in0 + imm2).astype(np.float32))
    # p = (h*c + C0)*c + C1   (tail of deg-4 Horner; in0=c, in1=h)
    mk("ANT_TP2B", (Src1 * Src0 + C0) * Src0 + C1,
       lambda in0, in1, s0, s1, imm2:
           ((in1.astype(np.float32) * in0 + s0) * in0 + s1).astype(np.float32))
    # (C0*a)^2 + (C1*b)^2
    _qa = (Src0 * C0) * (Src0 * C0)
    _qb = (Src1 * C1) * (Src1 * C1)
    mk("ANT_Q2", _qa + _qb,
       lambda in0, in1, s0, s1, imm2:
           ((s0 * in0.astype(np.float32)) ** 2 + (s1 * in1) ** 2)
           .astype(np.float32))
    # C0 * (a+b)^7 + C1  (fused chroma-sum sigmoid input)
    _ss = Src0 + Src1
    _t2 = _ss * _ss
    _t4 = _t2 * _t2
    mk("ANT_S7C2", (_t4 * _t2 * _ss) * C0 + C1,
       lambda in0, in1, s0, s1, imm2:
           (s0 * (in0.astype(np.float64) + in1) ** 7 + s1).astype(np.float32))
    # (a*b)^2
    _ab = Src0 * Src1
    mk("ANT_TSQ", _ab * _ab,
       lambda in0, in1, s0, s1, imm2:
           ((in0.astype(np.float32) * in1) ** 2).astype(np.float32))
    # C0 * s^7 + C1
    _s2 = Src0 * Src0
    _s4 = _s2 * _s2
    _s7 = _s4 * _s2 * Src0
    mk("ANT_S7C", _s7 * C0 + C1,
       lambda in0, in1, s0, s1, imm2:
           (s0 * in0.astype(np.float64) ** 7 + s1).astype(np.float32))
    # C0*a + C1*b
    mk("ANT_DOT2", Src0 * C0 + Src1 * C1,
       lambda in0, in1, s0, s1, imm2:
           (s0 * in0.astype(np.float32) + s1 * in1).astype(np.float32))
    # e * (K1 + K3 e^2 + K5 e^4): sin(K1*e) odd poly
    _e2 = Src0 * Src0
    mk("ANT_SINP", ((C0 * _e2 + C1) * _e2 + C2) * Src0,
       lambda in0, in1, s0, s1, imm2:
           (((s0 * in0.astype(np.float32) ** 2 + s1) * in0 ** 2 + imm2) * in0)
           .astype(np.float32))
    # a^2 + b^2
    mk("ANT_FQ1", Src0 * Src0 + Src1 * Src1,
       lambda in0, in1, s0, s1, imm2:
           (in0.astype(np.float32) ** 2 + in1.astype(np.float32) ** 2)
           .astype(np.float32))
    # a^2 + b
    mk("ANT_FQ2", Src0 * Src0 + Src1,
       lambda in0, in1, s0, s1, imm2:
           (in0.astype(np.float32) ** 2 + in1).astype(np.float32))
    # relu(a + b)
    mk("ANT_FSR", relu(Src0 + Src1),
       lambda in0, in1, s0, s1, imm2:
           np.maximum(in0.astype(np.float32) + in1, 0.0).astype(np.float32))
    # ((a*C0)*b + 1) * C1   (S-scale with folded output scale)
    mk("ANT_AFFS", ((Src0 * C0) * Src1 + One) * C1,
       lambda in0, in1, s0, s1, imm2:
           ((1.0 + s0 * in0.astype(np.float32) * in1) * s1).astype(np.float32))
    return _OPS


_BIASES = [0.055 / 1.055, -66.0, 20.0, -float(KG), 1.0, float(np.log(2.0))]


def _reg_consts(nc, values):
    for v in values:
        v = float(v)
        if (F32, v) not in nc.const_aps.aps:
            t = nc.alloc_sbuf_tensor(f"constf32_{repr(v)}", [128, 1], F32)
            nc.gpsimd.memset(t.ap(), v)
            nc.const_aps.aps[(F32, v)] = t.ap()
    nc.all_engine_barrier()


# tt ops that may be offloaded to GPSIMD (plain add/sub/mult only)
GP_OPS = {"m1", "m2", "m3", "v1", "v2", "tu", "Tt", "Wa", "Wb", "sfy", "ct",
          "dfy", "dCp", "cbb", "sbb", "rr"}


def build_nc(use_gp=True):
    ops = _register_custom_ops()
    POLY3, POLY2, TP2B = ops["ANT_POLY3"], ops["ANT_POLY2"], ops["ANT_TP2B"]
    Q2, S7C = ops["ANT_Q2"], ops["ANT_S7C"]
    DOT2, SINP = ops["ANT_DOT2"], ops["ANT_SINP"]
    FQ1, FQ2, FSR = ops["ANT_FQ1"], ops["ANT_FQ2"], ops["ANT_FSR"]
    AFFS = ops["ANT_AFFS"]
    S7C2, TSQ = ops["ANT_S7C2"], ops["ANT_TSQ"]

    nc = bacc.Bacc("TRN2", target_bir_lowering=False, debug=False)
    _reg_consts(nc, _BIASES)
    A = nc.scalar
    V = nc.vector
    Gp = nc.gpsimd

    shp = [IMGS_PER_CORE, 3, ROWS_PER_IMG, NCHUNK, S]
    x_d = nc.dram_tensor("x", shp, F32, kind="ExternalInput").ap()
    y_d = nc.dram_tensor("y", shp, F32, kind="ExternalInput").ap()
    out_d = nc.dram_tensor("out", [P, 1], F32, kind="ExternalOutput").ap()

    with tile.TileContext(nc) as tc, ExitStack() as ctx:
        pool = ctx.enter_context(tc.tile_pool(name="main", bufs=1))

        def TL(tag, w, dt):
            return pool.tile([P, w], dt, tag=tag, name=tag)

        # broadcast-constant [P,1] tiles (POLY3 d0 via C3-spill)
        def bctile(tag, val):
            t = pool.tile([P, 1], F32, tag=tag, name=tag)
            nc.gpsimd.memset(t[:], float(val))
            return t

        bc_lin = bctile("bc_lin", LINC[3])
        bc_g = bctile("bc_g", GC[3])
        bc_rc = bctile("bc_rc", SQ1Z[3])
        bc_tq = bctile("bc_tq", Q3_[3])
        nc.all_engine_barrier()

        acc = pool.tile([P, NCHUNK], F32, tag="acc", name="acc")

        NTMPF = 3      # rotating [P,S] f32 scratch
        NTMPB = 7      # rotating [P,S] bf16 scratch
        tmpi = [0, 0]

        def tmpf():
            t = TL(f"tmpf{tmpi[0] % NTMPF}", S, F32)
            tmpi[0] += 1
            return t

        def tmpb():
            t = TL(f"tmpb{tmpi[1] % NTMPB}", S, BF16)
            tmpi[1] += 1
            return t

        def emit_tt(name, out, a, b, op):
            if use_gp and name in GP_OPS:
                Gp.tensor_tensor(out[:], a, b, op)
            else:
                V.tensor_tensor(out[:], a, b, op)

        for k in range(NCHUNK):
            # ---- DMA input: rgb mega tile [P, 6144] = 3ch x (x|y) ----
            # alternating tag: chunk k+1 DMA overlaps chunk k compute
            rgb = TL(f"mega_rgb{k % 2}", M, F32)
            for c in range(3):
                for im in range(IMGS_PER_CORE):
                    pr = slice(im * ROWS_PER_IMG, (im + 1) * ROWS_PER_IMG)
                    nc.sync.dma_start(rgb[pr, c * D:c * D + S],
                                      x_d[im, c, :, k, :])
                    nc.sync.dma_start(rgb[pr, c * D + S:(c + 1) * D],
                                      y_d[im, c, :, k, :])

            # ---- linearize: deg-3 poly per channel (finer overlap) ----
            lin = TL("mega_lin", M, BF16)
            for c in range(3):
                V._custom_dve(POLY3, out=lin[:, c * D:(c + 1) * D],
                              in0=rgb[:, c * D:(c + 1) * D], in1=bc_lin[:],
                              s0=float(LINC[0]), s1=float(LINC[1]),
                              imm2=float(LINC[2]))

            # ---- xyz mix (per comp: DOT2 + stt) ----
            X = TL("mega_x", M, BF16)
            for kk in range(3):
                lr = lin[:, 0:D]
                lg = lin[:, D:2 * D]
                lb = lin[:, 2 * D:3 * D]
                t0 = TL("dT0", D, BF16)
                V._custom_dve(DOT2, out=t0[:], in0=lr, in1=lg,
                              s0=float(MW[kk, 0]), s1=float(MW[kk, 1]))
                V.scalar_tensor_tensor(X[:, kk * D:(kk + 1) * D], lb,
                                       float(MW[kk, 2]), t0[:],
                                       ALU.mult, ALU.add)

            # ---- cbrt via Ln/Exp per comp (one table set) ----
            LX = TL(f"mega_rgb{k % 2}", M, F32)  # rgb dead after lin poly
            f = TL("mega_f", M, BF16)
            for c in range(3):
                sl = slice(c * D, (c + 1) * D)
                A.activation(LX[:, sl], X[:, sl], AF.Ln)
                A.activation(f[:, sl], LX[:, sl], AF.Exp,
                             scale=float(1.0 / 3.0))

            fx = f[:, 0:D]
            fy = f[:, D:2 * D]
            fz = f[:, 2 * D:3 * D]
            fy1 = f[:, D:D + S]
            fy2 = f[:, D + S:2 * D]

            alpha = TL("alpha", D, BF16)
            emit_tt("alpha", alpha, fx, fy, ALU.subtract)
            beta = TL("beta", D, BF16)
            emit_tt("beta", beta, fy, fz, ALU.subtract)
            be1 = beta[:, 0:S]
            be2 = beta[:, S:D]

            # ---- pre-G chroma (exact, ACT sqrt) + G sigmoid ----
            qpre = TL("dP", D, F32)
            V._custom_dve(Q2, out=qpre[:], in0=alpha[:], in1=beta[:],
                          s0=5.0, s1=2.0)
            A.activation(qpre[:], qpre[:], AF.Ln)
            cpw = TL("dW", D, BF16)
            A.activation(cpw[:], qpre[:], AF.Exp, scale=0.5)
            z7 = tmpf()
            V._custom_dve(S7C2, out=z7[:], in0=cpw[:, 0:S], in1=cpw[:, S:D],
                          s0=128.0, s1=1.0)
            zr = tmpf()
            V.reciprocal_approx_fast(zr[:], z7[:])
            lz = tmpf()
            A.activation(lz[:], zr[:], AF.Ln, scale=-1.0, bias=1.0)
            wg = tmpb()
            A.activation(wg[:], lz[:], AF.Exp, scale=0.5)
            g1 = TL("g1", S, BF16)
            V.tensor_scalar(g1[:], wg[:], -0.5, 1.5, ALU.mult, ALU.add)

            # ---- post-G chroma ----
            ap = TL("ap", D, BF16)
            V.tensor_tensor(ap[:, 0:S], alpha[:, 0:S], g1[:], ALU.mult)
            V.tensor_tensor(ap[:, S:D], alpha[:, S:D], g1[:], ALU.mult)
            qp = TL("dQ", D, F32)
            V._custom_dve(Q2, out=qp[:], in0=ap[:], in1=beta[:],
                          s0=5.0, s1=2.0)
            A.activation(qp[:], qp[:], AF.Ln)
            yq = TL("yq", D, BF16)
            A.activation(yq[:], qp[:], AF.Exp, scale=-0.5)
            cp = TL("cp", D, BF16)
            A.activation(cp[:], qp[:], AF.Exp, scale=0.5)

            dCp = TL("dCp", S, BF16)
            emit_tt("dCp", dCp, cp[:, S:D], cp[:, 0:S], ALU.subtract)
            Scp = TL("Scp", S, F32)
            V.tensor_tensor(Scp[:], cp[:, 0:S], cp[:, S:D], ALU.add)
            SCf = tmpf()
            V.tensor_scalar(SCf[:], Scp[:], 0.0225, 0.01, ALU.mult, ALU.add)
            lsc = tmpf()
            A.activation(lsc[:], SCf[:], AF.Ln)
            iC = TL("iC", S, F32)
            A.activation(iC[:], lsc[:], AF.Exp, scale=-1.0)

            # ---- RC sigmoid (same shape as G) ----
            z7c = tmpf()
            V._custom_dve(S7C, out=z7c[:], in0=Scp[:], s0=128.0, s1=1.0)
            zrc = tmpf()
            V.reciprocal_approx_fast(zrc[:], z7c[:])
            lzc = tmpf()
            A.activation(lzc[:], zrc[:], AF.Ln, scale=-1.0, bias=1.0)
            rsqC = TL("rsqC", S, BF16)
            A.activation(rsqC[:], lzc[:], AF.Exp, scale=0.5)

            # ---- SL ----
            sfy = TL("sfy", S, BF16)
            emit_tt("sfy", sfy, fy1, fy2, ALU.add)
            dfy = TL("dfy", S, BF16)
            emit_tt("dfy", dfy, fy2, fy1, ALU.subtract)
            L50 = TL("L50", S, F32)
            A.activation(L50[:], sfy[:], AF.Square, scale=58.0, bias=-66.0)
            l20 = tmpf()
            A.activation(l20[:], L50[:], AF.Ln, bias=20.0)
            rsq20 = tmpb()
            A.activation(rsq20[:], l20[:], AF.Exp, scale=-0.5)
            SLf = tmpf()
            V._custom_dve(AFFS, out=SLf[:], in0=L50[:], in1=rsq20[:],
                          s0=0.015, s1=float(1.0 / 116.0))
            iL = TL("iL", S, F32)
            V.reciprocal_approx_fast(iL[:], SLf[:])

            # ---- unit chroma vectors + bisector ----
            ca = TL("dA", D, BF16)
            V.scalar_tensor_tensor(ca[:], ap[:], 5.0, yq[:], ALU.mult,
                                   ALU.mult)
            cbt = TL("dB", D, BF16)
            V.scalar_tensor_tensor(cbt[:], beta[:], 2.0, yq[:], ALU.mult,
                                   ALU.mult)
            Wa = TL("Wa", S, BF16)
            emit_tt("Wa", Wa, ca[:, 0:S], ca[:, S:D], ALU.add)
            Wb = TL("Wb", S, BF16)
            emit_tt("Wb", Wb, cbt[:, 0:S], cbt[:, S:D], ALU.add)
            n2 = tmpf()
            V._custom_dve(Q2, out=n2[:], in0=Wa[:], in1=Wb[:], s0=1.0, s1=1.0)
            ln2 = tmpf()
            A.activation(ln2[:], n2[:], AF.Ln)
            rn = tmpb()
            A.activation(rn[:], ln2[:], AF.Exp, scale=-0.5)
            cbb = TL("cbb", S, BF16)
            emit_tt("cbb", cbb, Wa[:], rn[:], ALU.mult)
            sbb = TL("sbb", S, BF16)
            emit_tt("sbb", sbb, Wb[:], rn[:], ALU.mult)

            # ---- T = P4(cb) + sb*Q3(cb) ----
            q3t = tmpb()
            V._custom_dve(POLY3, out=q3t[:], in0=cbb[:], in1=bc_tq[:],
                          s0=float(Q3_[0]), s1=float(Q3_[1]),
                          imm2=float(Q3_[2]))
            hh = tmpb()
            V._custom_dve(POLY2, out=hh[:], in0=cbb[:],
                          s0=float(E4), s1=float(E3), imm2=float(E2))
            pp = tmpb()
            V._custom_dve(TP2B, out=pp[:], in0=cbb[:], in1=hh[:],
                          s0=float(E1), s1=float(E0))
            tu = tmpb()
            emit_tt("tu", tu, sbb[:], q3t[:], ALU.mult)
            Tt = tmpb()
            emit_tt("Tt", Tt, pp[:], tu[:], ALU.add)
            SHf = tmpf()
            V._custom_dve(AFFS, out=SHf[:], in0=Scp[:], in1=Tt[:],
                          s0=0.75, s1=0.01)
            lsh = tmpf()
            A.activation(lsh[:], SHf[:], AF.Ln)
            iH = TL("iH", S, F32)
            A.activation(iH[:], lsh[:], AF.Exp, scale=-1.0)

            # ---- RT: gaussian via cos identity + sin poly ----
            cd = tmpb()
            V._custom_dve(DOT2, out=cd[:], in0=cbb[:], in1=sbb[:],
                          s0=float(COS275), s1=float(SIN275))
            ee = tmpb()
            A.activation(ee[:], cd[:], AF.Exp, scale=float(KG),
                         bias=-float(KG))
            s2d = tmpb()
            V._custom_dve(SINP, out=s2d[:], in0=ee[:],
                          s0=float(K5), s1=float(K3), imm2=float(K1))
            rr = TL("rr", S, BF16)
            emit_tt("rr", rr, s2d[:], rsqC[:], ALU.mult)

            # ---- signed dHp = 2*sqrt(c1'c2') * sin(dh/2) ----
            # sin(dh/2) = (u1 x u2) * rn exactly (sign automatic)
            lqs = tmpf()
            V.tensor_tensor(lqs[:], qp[:, 0:S], qp[:, S:D], ALU.add)
            sq2 = tmpb()
            A.activation(sq2[:], lqs[:], AF.Exp, scale=0.25,
                         bias=float(np.log(2.0)))
            v1 = tmpb()
            emit_tt("v1", v1, ca[:, 0:S], cbt[:, S:D], ALU.mult)
            v2 = tmpb()
            emit_tt("v2", v2, cbt[:, 0:S], ca[:, S:D], ALU.mult)
            cr = tmpf()
            V.tensor_tensor(cr[:], v1[:], v2[:], ALU.subtract)
            sdh = tmpb()
            V.tensor_tensor(sdh[:], cr[:], rn[:], ALU.mult)
            hn = tmpb()
            V.tensor_tensor(hn[:], sdh[:], sq2[:], ALU.mult)

            # ---- assemble F ----
            tL2 = tmpb()
            V._custom_dve(TSQ, out=tL2[:], in0=dfy[:], in1=iL[:])
            tC = tmpb()
            emit_tt("tC", tC, dCp[:], iC[:], ALU.mult)
            tH = tmpb()
            emit_tt("tH", tH, hn[:], iH[:], ALU.mult)
            F1 = tmpb()
            V._custom_dve(FQ2, out=F1[:], in0=tC[:], in1=tL2[:])
            F2 = tmpb()
            V._custom_dve(FQ2, out=F2[:], in0=tH[:], in1=F1[:])
            ct = tmpb()
            emit_tt("ct", ct, tC[:], tH[:], ALU.mult)
            ctr = tmpb()
            V.scalar_tensor_tensor(ctr[:], rr[:], -2.0, ct[:], ALU.mult,
                                   ALU.mult)
            FS = tmpf()
            V._custom_dve(FSR, out=FS[:], in0=F2[:], in1=ctr[:])
            lF = tmpf()
            A.activation(lF[:], FS[:], AF.Ln)
            dE = tmpf()
            A.activation(dE[:], lF[:], AF.Exp, scale=0.5,
                         accum_out=acc[:, k:k + 1])

        accsum = pool.tile([P, 1], F32, tag="accsum", name="accsum")
        V.tensor_reduce(accsum[:], acc[:], mybir.AxisListType.X, ALU.add)
        nc.sync.dma_start(out_d[:], accsum[:])

    nc.compile()
    return nc


def _get_nc():
    if "nc" not in _NC_CACHE:
        _NC_CACHE["nc"] = build_nc(use_gp=bool(int(
            os.environ.get("COLOR_GP", "1"))))
    return _NC_CACHE["nc"]


def kernel(x: np.ndarray, y: np.ndarray) -> np.ndarray:
    assert x.shape == (32, 3, 512, 512) and y.shape == (32, 3, 512, 512)
    nc = _get_nc()
    shp = (IMGS_PER_CORE, 3, ROWS_PER_IMG, NCHUNK, S)
    xs = np.ascontiguousarray(x, dtype=np.float32)
    ys = np.ascontiguousarray(y, dtype=np.float32)
    in_maps = []
    for c in range(NCORE):
        xi = xs[c * IMGS_PER_CORE:(c + 1) * IMGS_PER_CORE].reshape(shp)
        yi = ys[c * IMGS_PER_CORE:(c + 1) * IMGS_PER_CORE].reshape(shp)
        in_maps.append({"x": xi, "y": yi})
    trace = bool(int(os.environ.get("COLOR_TRACE", "0")))
    res = run_bass_kernel_spmd(nc, in_maps, core_ids=list(range(NCORE)),
                               trace=trace)
    _NC_CACHE["last_results"] = res
    total = np.float64(0.0)
    for c in range(NCORE):
        total += np.float64(res.results[c]["out"].sum())
    npix = 32 * 512 * 512
    return np.float32(total / npix / 100.0)


# revision 34
# speedup vs baseline: 1.0222x; 1.0103x over previous
"""CIEDE2000 ColorLoss kernel v2.1 for Trainium2, 8 NeuronCores, data-parallel.

Full inputs x, y: [32, 3, 512, 512] f32 NCHW in [0, 1).
Output: scalar f32 = mean(ciede2000(rgb2lab(x), rgb2lab(y))) / 100.

v2 design (vs the v1 baseline):
  - ONE ACT table set (natural_log_exp): all roots/powers as Ln/Exp pairs,
    no Arctan/Sin tables -> no ACT_TABLE_LOAD thrash.
  - No GPSIMD tensor_scalar (19.7us each on HW); GPSIMD only runs plain
    tensor_tensor add/sub/mult offloads.
  - Hue handled vectorially (no angles): (cos h, sin h) unit vectors, the
    CIEDE2000 hbar wrap rule == bisector of the smaller arc, so
    (cos hbar, sin hbar) = normalize(u1+u2). T expands exactly as
    P4(cos hbar) + sin hbar * Q3(cos hbar).
  - dHp via dHp^2 = 2*(c1'c2' - 25a1'a2' - 4b1b2), sign from the cross
    product; dtheta gaussian via cos(hbar-275deg) small-angle identity.
  - sRGB linearization: single fused deg-3 poly (custom DVE op), dark-branch
    dropped (validated: end-to-end rel err ~1e-4).
  - G / RC chroma sigmoids: s^7 power chain + approx reciprocal + deg-3 poly
    of sqrt(1-z), all on DVE.
  - bf16 storage for most intermediates (DVE 2x modes); fp32 where bit
    tricks (reciprocal seed) or accuracy demand it.
  - 12 new fused custom DVE ops registered at import (runtime extension of
    concourse.dve_ops.OPS).
"""
import os
import re
import sys

sys.path.insert(0, "/opt/trn_rl_repo")

import numpy as np
import concourse.bacc as bacc
import concourse.tile as tile
import concourse.mybir as mybir
from concourse.bass_utils import run_bass_kernel_spmd
from contextlib import ExitStack

F32 = mybir.dt.float32
BF16 = mybir.dt.bfloat16
AF = mybir.ActivationFunctionType
ALU = mybir.AluOpType

P = 128
S = 1024          # chunk free dim (pixel pairs per partition-col block)
D = 2 * S         # both-images width
M = 3 * D         # 3 channels / comps width
NCHUNK = 8
NCORE = 8
IMGS_PER_CORE = 4
ROWS_PER_IMG = 32

# ---------------- constants ----------------
_M = np.array([[0.412453, 0.357580, 0.180423],
               [0.212671, 0.715160, 0.072169],
               [0.019334, 0.119193, 0.950227]], dtype=np.float64)
_W = np.array([0.95047, 1.0, 1.08883], dtype=np.float64)
MW = (_M / _W[:, None])            # [3,3] row k = xyz_k coeffs

_d = np.deg2rad
COS30, SIN30 = np.cos(_d(30)), np.sin(_d(30))
COS6, SIN6 = np.cos(_d(6)), np.sin(_d(6))
COS63, SIN63 = np.cos(_d(63)), np.sin(_d(63))
COS275, SIN275 = np.cos(_d(275)), np.sin(_d(275))

# T = P4(cb) + sb*Q3(cb)  (exact 4-harmonic expansion)
E4 = -1.6 * COS63
E3 = 1.28 * COS6
E2 = 0.48 + 1.6 * COS63
E1 = -0.17 * COS30 - 0.96 * COS6
E0 = 1.0 - 0.24 - 0.20 * COS63
Q3_ = [-1.6 * SIN63, -1.28 * SIN6, 0.8 * SIN63, 0.32 * SIN6 - 0.085]

K1 = 1.0471976          # 60deg in rad (2*dtheta = K1 * e)
K3 = -K1 ** 3 / 6.0
K5 = K1 ** 5 / 120.0
KG = 2.0 * (180.0 / np.pi) ** 2 / 625.0   # 10.50499: (hbar-275)^2/25^2 ~ KG*(1-cos)

PWA, PWB = 0.9398, 0.3893   # 1-seg PWL hypot coefficients


def _fit_poly(f, lo, hi, deg, w=None, n=20001):
    x = np.linspace(lo, hi, n)
    yv = f(x)
    Wt = np.ones_like(x) if w is None else w(x)
    V = np.vander(x, deg + 1)
    c, *_ = np.linalg.lstsq(V * Wt[:, None], yv * Wt, rcond=None)
    return c            # highest power first


LINC = _fit_poly(lambda x: ((x + 0.055) / 1.055) ** 2.4, 0.0, 1.0, 3)
SQ1Z = _fit_poly(lambda z: np.sqrt(1.0 - z), 0.0, 1.0, 3,
                 w=lambda z: 1.0 / np.sqrt(0.005 + z))
GC = -0.5 * SQ1Z                   # g1 = 1.5 - 0.5*p(z)
GC[-1] += 1.5

_NC_CACHE = {}

# Force every ACT op onto the combined ln+exp table set: hide all other
# sets from the table-load chooser (indices preserved; set 6 has ln, exp,
# square, sign, relu, abs).
import concourse.bacc as _bacc_mod
from concourse.hw_specs import get_activation_tables as _gat_orig

def _gat_one_set(arch):
    tabs = _gat_orig(arch)
    keep = "natural_log_exp_and_others"
    return {n: (f if n == keep else set()) for n, f in tabs.items()}

_bacc_mod.get_activation_tables = _gat_one_set

# ---------------- custom DVE ops ----------------
_OPS = {}


def _register_custom_ops():
    if _OPS:
        return _OPS
    import concourse.dve_ops as _dm
    prior = {o.name: o for o in _dm.OPS if o.name.startswith("ANT_")}
    if prior:            # already registered in this process (re-import)
        _OPS.update(prior)
        return _OPS
    from concourse.dve_spec import (
        Spec, Src0, Src1, C0, C1, C2, C3, Zero, One, relu, maxx, minn, lower,
        _spill_c3_to_src1,
    )
    import concourse.dve_ops as dmod

    def mk(name, body, ref, spill=False):
        spec = Spec(body=_spill_c3_to_src1(body) if spill else body,
                    reference=ref)
        row = max(dmod._SUB_OPCODE_FOR_NAME.values()) + 1
        assert row < 0x20, "custom DVE opcode rows exhausted"
        dmod._SUB_OPCODE_FOR_NAME[name] = row
        op = dmod.DveOp(name, spec, subdim=False, uops_sha={})
        for ver in ("v3", "v4"):
            try:
                op.compile(ver)
            except ValueError as e:
                m = re.search(r"([0-9a-f]{16})\s*≠\s*pinned", str(e))
                if not m:
                    raise
                op.uops_sha[ver] = m.group(1)
                op.compile(ver)
            except Exception:
                pass        # ver not supported; fine if unused
        dmod.OPS.append(op)
        dmod.CUSTOM_DVE_SPECS[name] = op.spec
        _OPS[name] = op
        return op

    # deg-3 Horner; d0 spilled to Src1 ([P,1] broadcast tile)
    mk("ANT_POLY3", ((C0 * Src0 + C1) * Src0 + C2) * Src0 + C3,
       lambda in0, in1, s0, s1, imm2:
           (((s0 * in0.astype(np.float32) + s1) * in0 + imm2) * in0
            + in1).astype(np.float32), spill=True)
    # deg-2 Horner
    mk("ANT_POLY2", (C0 * Src0 + C1) * Src0 + C2,
       lambda in0, in1, s0, s1, imm2:
           ((s0 * in0.astype(np.float32) + s1) * in0 + imm2).astype(np.float32))
    # p = (h*c + C0)*c + C1   (tail of deg-4 Horner; in0=c, in1=h)
    mk("ANT_TP2B", (Src1 * Src0 + C0) * Src0 + C1,
       lambda in0, in1, s0, s1, imm2:
           ((in1.astype(np.float32) * in0 + s0) * in0 + s1).astype(np.float32))
    # (C0*a)^2 + (C1*b)^2
    _qa = (Src0 * C0) * (Src0 * C0)
    _qb = (Src1 * C1) * (Src1 * C1)
    mk("ANT_Q2", _qa + _qb,
       lambda in0, in1, s0, s1, imm2:
           ((s0 * in0.astype(np.float32)) ** 2 + (s1 * in1) ** 2)
           .astype(np.float32))
    # C0 * (a+b)^7 + C1  (fused chroma-sum sigmoid input)
    _ss = Src0 + Src1
    _t2 = _ss * _ss
    _t4 = _t2 * _t2
    mk("ANT_S7C2", (_t4 * _t2 * _ss) * C0 + C1,
       lambda in0, in1, s0, s1, imm2:
           (s0 * (in0.astype(np.float64) + in1) ** 7 + s1).astype(np.float32))
    # (a*b)^2
    _ab = Src0 * Src1
    mk("ANT_TSQ", _ab * _ab,
       lambda in0, in1, s0, s1, imm2:
           ((in0.astype(np.float32) * in1) ** 2).astype(np.float32))
    # C0 * s^7 + C1
    _s2 = Src0 * Src0
    _s4 = _s2 * _s2
    _s7 = _s4 * _s2 * Src0
    mk("ANT_S7C", _s7 * C0 + C1,
       lambda in0, in1, s0, s1, imm2:
           (s0 * in0.astype(np.float64) ** 7 + s1).astype(np.float32))
    # C0*a + C1*b
    mk("ANT_DOT2", Src0 * C0 + Src1 * C1,
       lambda in0, in1, s0, s1, imm2:
           (s0 * in0.astype(np.float32) + s1 * in1).astype(np.float32))
    # e * (K1 + K3 e^2 + K5 e^4): sin(K1*e) odd poly
    _e2 = Src0 * Src0
    mk("ANT_SINP", ((C0 * _e2 + C1) * _e2 + C2) * Src0,
       lambda in0, in1, s0, s1, imm2:
           (((s0 * in0.astype(np.float32) ** 2 + s1) * in0 ** 2 + imm2) * in0)
           .astype(np.float32))
    # a^2 + b^2
    mk("ANT_FQ1", Src0 * Src0 + Src1 * Src1,
       lambda in0, in1, s0, s1, imm2:
           (in0.astype(np.float32) ** 2 + in1.astype(np.float32) ** 2)
           .astype(np.float32))
    # a^2 + b
    mk("ANT_FQ2", Src0 * Src0 + Src1,
       lambda in0, in1, s0, s1, imm2:
           (in0.astype(np.float32) ** 2 + in1).astype(np.float32))
    # relu(a + b)
    mk("ANT_FSR", relu(Src0 + Src1),
       lambda in0, in1, s0, s1, imm2:
           np.maximum(in0.astype(np.float32) + in1, 0.0).astype(np.float32))
    # ((a*C0)*b + 1) * C1   (S-scale with folded output scale)
    mk("ANT_AFFS", ((Src0 * C0) * Src1 + One) * C1,
       lambda in0, in1, s0, s1, imm2:
           ((1.0 + s0 * in0.astype(np.float32) * in1) * s1).astype(np.float32))
    return _OPS


_BIASES = [0.055 / 1.055, -66.0, 20.0, -float(KG), 1.0, float(np.log(2.0)),
           float(np.log(20.0))]


def _reg_consts(nc, values):
    for v in values:
        v = float(v)
        if (F32, v) not in nc.const_aps.aps:
            t = nc.alloc_sbuf_tensor(f"constf32_{repr(v)}", [128, 1], F32)
            nc.gpsimd.memset(t.ap(), v)
            nc.const_aps.aps[(F32, v)] = t.ap()
    nc.all_engine_barrier()


# tt ops that may be offloaded to GPSIMD (plain add/sub/mult only)
GP_OPS = {"m1", "m2", "m3", "v1", "v2", "tu", "Tt", "Wa", "Wb", "sfy",
          "dfy", "dCp", "cbb", "sbb", "rr"}


def build_nc(use_gp=True):
    ops = _register_custom_ops()
    POLY3, POLY2, TP2B = ops["ANT_POLY3"], ops["ANT_POLY2"], ops["ANT_TP2B"]
    Q2, S7C = ops["ANT_Q2"], ops["ANT_S7C"]
    DOT2, SINP = ops["ANT_DOT2"], ops["ANT_SINP"]
    FQ1, FQ2, FSR = ops["ANT_FQ1"], ops["ANT_FQ2"], ops["ANT_FSR"]
    AFFS = ops["ANT_AFFS"]
    S7C2, TSQ = ops["ANT_S7C2"], ops["ANT_TSQ"]

    nc = bacc.Bacc("TRN2", target_bir_lowering=False, debug=False)
    _reg_consts(nc, _BIASES)
    A = nc.scalar
    V = nc.vector
    Gp = nc.gpsimd

    shp = [IMGS_PER_CORE, 3, ROWS_PER_IMG, NCHUNK, S]
    x_d = nc.dram_tensor("x", shp, F32, kind="ExternalInput").ap()
    y_d = nc.dram_tensor("y", shp, F32, kind="ExternalInput").ap()
    out_d = nc.dram_tensor("out", [P, 1], F32, kind="ExternalOutput").ap()

    with tile.TileContext(nc) as tc, ExitStack() as ctx:
        pool = ctx.enter_context(tc.tile_pool(name="main", bufs=1))

        def TL(tag, w, dt):
            return pool.tile([P, w], dt, tag=tag, name=tag)

        # broadcast-constant [P,1] tiles (POLY3 d0 via C3-spill)
        def bctile(tag, val):
            t = pool.tile([P, 1], F32, tag=tag, name=tag)
            nc.gpsimd.memset(t[:], float(val))
            return t

        bc_lin = bctile("bc_lin", LINC[3])
        bc_g = bctile("bc_g", GC[3])
        bc_rc = bctile("bc_rc", SQ1Z[3])
        bc_tq = bctile("bc_tq", Q3_[3])
        nc.all_engine_barrier()

        acc = pool.tile([P, NCHUNK], F32, tag="acc", name="acc")

        NTMPF = 3      # rotating [P,S] f32 scratch
        NTMPB = 7      # rotating [P,S] bf16 scratch
        tmpi = [0, 0]

        def tmpf():
            t = TL(f"tmpf{tmpi[0] % NTMPF}", S, F32)
            tmpi[0] += 1
            return t

        def tmpb():
            t = TL(f"tmpb{tmpi[1] % NTMPB}", S, BF16)
            tmpi[1] += 1
            return t

        def emit_tt(name, out, a, b, op):
            if use_gp and name in GP_OPS:
                Gp.tensor_tensor(out[:], a, b, op)
            else:
                V.tensor_tensor(out[:], a, b, op)

        for k in range(NCHUNK):
            # ---- DMA input: rgb mega tile [P, 6144] = 3ch x (x|y) ----
            # alternating tag: chunk k+1 DMA overlaps chunk k compute
            rgb = TL(f"mega_rgb{k % 2}", M, F32)
            for c in range(3):
                for im in range(IMGS_PER_CORE):
                    pr = slice(im * ROWS_PER_IMG, (im + 1) * ROWS_PER_IMG)
                    nc.sync.dma_start(rgb[pr, c * D:c * D + S],
                                      x_d[im, c, :, k, :])
                    nc.sync.dma_start(rgb[pr, c * D + S:(c + 1) * D],
                                      y_d[im, c, :, k, :])

            # ---- linearize: deg-3 poly per channel (finer overlap) ----
            lin = TL("mega_lin", M, BF16)
            for c in range(3):
                V._custom_dve(POLY3, out=lin[:, c * D:(c + 1) * D],
                              in0=rgb[:, c * D:(c + 1) * D], in1=bc_lin[:],
                              s0=float(LINC[0]), s1=float(LINC[1]),
                              imm2=float(LINC[2]))

            # ---- xyz mix (per comp: DOT2 + stt) ----
            X = TL("mega_x", M, BF16)
            for kk in range(3):
                lr = lin[:, 0:D]
                lg = lin[:, D:2 * D]
                lb = lin[:, 2 * D:3 * D]
                t0 = TL("dT0", D, BF16)
                V._custom_dve(DOT2, out=t0[:], in0=lr, in1=lg,
                              s0=float(MW[kk, 0]), s1=float(MW[kk, 1]))
                V.scalar_tensor_tensor(X[:, kk * D:(kk + 1) * D], lb,
                                       float(MW[kk, 2]), t0[:],
                                       ALU.mult, ALU.add)

            # ---- cbrt via Ln/Exp per comp (one table set) ----
            LX = TL(f"mega_rgb{k % 2}", M, F32)  # rgb dead after lin poly
            f = TL("mega_f", M, BF16)
            for c in range(3):
                sl = slice(c * D, (c + 1) * D)
                A.activation(LX[:, sl], X[:, sl], AF.Ln)
                A.activation(f[:, sl], LX[:, sl], AF.Exp,
                             scale=float(1.0 / 3.0))

            fx = f[:, 0:D]
            fy = f[:, D:2 * D]
            fz = f[:, 2 * D:3 * D]
            fy1 = f[:, D:D + S]
            fy2 = f[:, D + S:2 * D]

            alpha = TL("alpha", D, BF16)
            emit_tt("alpha", alpha, fx, fy, ALU.subtract)
            beta = TL("beta", D, BF16)
            emit_tt("beta", beta, fy, fz, ALU.subtract)
            be1 = beta[:, 0:S]
            be2 = beta[:, S:D]

            # ---- pre-G chroma (exact, ACT sqrt) + G sigmoid ----
            qpre = TL("dP", D, F32)
            V._custom_dve(Q2, out=qpre[:], in0=alpha[:], in1=beta[:],
                          s0=5.0, s1=2.0)
            A.activation(qpre[:], qpre[:], AF.Ln)
            cpw = TL("dW", D, BF16)
            A.activation(cpw[:], qpre[:], AF.Exp, scale=0.5)
            z7 = tmpf()
            V._custom_dve(S7C2, out=z7[:], in0=cpw[:, 0:S], in1=cpw[:, S:D],
                          s0=128.0, s1=1.0)
            zr = tmpf()
            V.reciprocal_approx_fast(zr[:], z7[:])
            lz = tmpf()
            A.activation(lz[:], zr[:], AF.Ln, scale=-1.0, bias=1.0)
            wg = tmpb()
            A.activation(wg[:], lz[:], AF.Exp, scale=0.5)
            g1 = TL("g1", S, BF16)
            V.tensor_scalar(g1[:], wg[:], -0.5, 1.5, ALU.mult, ALU.add)

            # ---- post-G chroma ----
            ap = TL("ap", D, BF16)
            V.tensor_tensor(ap[:, 0:S], alpha[:, 0:S], g1[:], ALU.mult)
            V.tensor_tensor(ap[:, S:D], alpha[:, S:D], g1[:], ALU.mult)
            qp = TL("dQ", D, F32)
            V._custom_dve(Q2, out=qp[:], in0=ap[:], in1=beta[:],
                          s0=5.0, s1=2.0)
            A.activation(qp[:], qp[:], AF.Ln)
            yq = TL("yq", D, BF16)
            A.activation(yq[:], qp[:], AF.Exp, scale=-0.5)
            cp = TL("cp", D, BF16)
            A.activation(cp[:], qp[:], AF.Exp, scale=0.5)

            dCp = TL("dCp", S, BF16)
            emit_tt("dCp", dCp, cp[:, S:D], cp[:, 0:S], ALU.subtract)
            Scp = TL("Scp", S, F32)
            V.tensor_tensor(Scp[:], cp[:, 0:S], cp[:, S:D], ALU.add)
            SCf = tmpf()
            V.tensor_scalar(SCf[:], Scp[:], 0.0225, 0.01, ALU.mult, ALU.add)
            lsc = tmpf()
            A.activation(lsc[:], SCf[:], AF.Ln)
            iC = TL("iC", S, F32)
            A.activation(iC[:], lsc[:], AF.Exp, scale=-1.0)

            # ---- RC sigmoid (same shape as G) ----
            z7c = tmpf()
            V._custom_dve(S7C, out=z7c[:], in0=Scp[:], s0=128.0, s1=1.0)
            zrc = tmpf()
            V.reciprocal_approx_fast(zrc[:], z7c[:])
            lzc = tmpf()
            A.activation(lzc[:], zrc[:], AF.Ln, scale=-1.0, bias=1.0)
            rsqC = TL("rsqC", S, BF16)
            A.activation(rsqC[:], lzc[:], AF.Exp, scale=0.5)

            # ---- SL ----
            sfy = TL("sfy", S, BF16)
            emit_tt("sfy", sfy, fy1, fy2, ALU.add)
            dfy = TL("dfy", S, BF16)
            emit_tt("dfy", dfy, fy2, fy1, ALU.subtract)
            L50 = TL("L50", S, F32)
            A.activation(L50[:], sfy[:], AF.Square, scale=58.0, bias=-66.0)
            l20 = tmpf()
            A.activation(l20[:], L50[:], AF.Ln, bias=20.0)
            rsq20 = tmpb()
            A.activation(rsq20[:], l20[:], AF.Exp, scale=-0.5)
            SLf = tmpf()
            V._custom_dve(AFFS, out=SLf[:], in0=L50[:], in1=rsq20[:],
                          s0=0.015, s1=float(1.0 / 116.0))
            iL = TL("iL", S, F32)
            V.reciprocal_approx_fast(iL[:], SLf[:])

            # ---- unit chroma vectors + bisector ----
            ca = TL("dA", D, BF16)
            V.scalar_tensor_tensor(ca[:], ap[:], 5.0, yq[:], ALU.mult,
                                   ALU.mult)
            cbt = TL("dB", D, BF16)
            V.scalar_tensor_tensor(cbt[:], beta[:], 2.0, yq[:], ALU.mult,
                                   ALU.mult)
            Wa = TL("Wa", S, BF16)
            emit_tt("Wa", Wa, ca[:, 0:S], ca[:, S:D], ALU.add)
            Wb = TL("Wb", S, BF16)
            emit_tt("Wb", Wb, cbt[:, 0:S], cbt[:, S:D], ALU.add)
            n2 = tmpf()
            V._custom_dve(Q2, out=n2[:], in0=Wa[:], in1=Wb[:], s0=1.0, s1=1.0)
            ln2 = tmpf()
            A.activation(ln2[:], n2[:], AF.Ln)
            rn = tmpb()
            A.activation(rn[:], ln2[:], AF.Exp, scale=-0.5)
            cbb = TL("cbb", S, BF16)
            emit_tt("cbb", cbb, Wa[:], rn[:], ALU.mult)
            sbb = TL("sbb", S, BF16)
            emit_tt("sbb", sbb, Wb[:], rn[:], ALU.mult)

            # ---- T = P4(cb) + sb*Q3(cb) ----
            q3t = tmpb()
            V._custom_dve(POLY3, out=q3t[:], in0=cbb[:], in1=bc_tq[:],
                          s0=float(Q3_[0]), s1=float(Q3_[1]),
                          imm2=float(Q3_[2]))
            hh = tmpb()
            V._custom_dve(POLY2, out=hh[:], in0=cbb[:],
                          s0=float(E4), s1=float(E3), imm2=float(E2))
            pp = tmpb()
            V._custom_dve(TP2B, out=pp[:], in0=cbb[:], in1=hh[:],
                          s0=float(E1), s1=float(E0))
            tu = tmpb()
            emit_tt("tu", tu, sbb[:], q3t[:], ALU.mult)
            Tt = tmpb()
            emit_tt("Tt", Tt, pp[:], tu[:], ALU.add)
            SHf = tmpf()
            V._custom_dve(AFFS, out=SHf[:], in0=Scp[:], in1=Tt[:],
                          s0=0.75, s1=0.01)
            lsh = tmpf()
            A.activation(lsh[:], SHf[:], AF.Ln)
            iH = TL("iH", S, F32)
            A.activation(iH[:], lsh[:], AF.Exp, scale=-1.0)

            # ---- RT: gaussian via cos identity + sin poly ----
            cd = tmpb()
            V._custom_dve(DOT2, out=cd[:], in0=cbb[:], in1=sbb[:],
                          s0=float(COS275), s1=float(SIN275))
            ee = tmpb()
            A.activation(ee[:], cd[:], AF.Exp, scale=float(KG),
                         bias=-float(KG))
            s2d = tmpb()
            V._custom_dve(SINP, out=s2d[:], in0=ee[:],
                          s0=float(K5), s1=float(K3), imm2=float(K1))
            rr = TL("rr", S, BF16)
            emit_tt("rr", rr, s2d[:], rsqC[:], ALU.mult)

            # ---- signed dHp = 2*sqrt(c1'c2') * sin(dh/2) ----
            # sin(dh/2) = (u1 x u2) * rn exactly (sign automatic)
            lqs = tmpf()
            V.tensor_tensor(lqs[:], qp[:, 0:S], qp[:, S:D], ALU.add)
            sq2 = tmpb()
            A.activation(sq2[:], lqs[:], AF.Exp, scale=-0.25,
                         bias=float(np.log(20.0)))
            v1 = tmpb()
            emit_tt("v1", v1, ap[:, 0:S], be2, ALU.mult)
            v2 = tmpb()
            emit_tt("v2", v2, ap[:, S:D], be1, ALU.mult)
            cr = tmpf()
            V.tensor_tensor(cr[:], v1[:], v2[:], ALU.subtract)
            sdh = tmpb()
            V.tensor_tensor(sdh[:], cr[:], rn[:], ALU.mult)
            hn = tmpb()
            V.tensor_tensor(hn[:], sdh[:], sq2[:], ALU.mult)

            # ---- assemble F ----
            tL2 = tmpb()
            V._custom_dve(TSQ, out=tL2[:], in0=dfy[:], in1=iL[:])
            tC = tmpb()
            emit_tt("tC", tC, dCp[:], iC[:], ALU.mult)
            tH = tmpb()
            emit_tt("tH", tH, hn[:], iH[:], ALU.mult)
            F1 = tmpb()
            V._custom_dve(FQ2, out=F1[:], in0=tC[:], in1=tL2[:])
            F2 = tmpb()
            V._custom_dve(FQ2, out=F2[:], in0=tH[:], in1=F1[:])
            ct = tmpb()
            emit_tt("ct", ct, tC[:], tH[:], ALU.mult)
            ctr = tmpb()
            V.scalar_tensor_tensor(ctr[:], rr[:], -2.0, ct[:], ALU.mult,
                                   ALU.mult)
            FS = tmpf()
            V._custom_dve(FSR, out=FS[:], in0=F2[:], in1=ctr[:])
            lF = tmpf()
            A.activation(lF[:], FS[:], AF.Ln)
            dE = tmpf()
            A.activation(dE[:], lF[:], AF.Exp, scale=0.5,
                         accum_out=acc[:, k:k + 1])

        accsum = pool.tile([P, 1], F32, tag="accsum", name="accsum")
        V.tensor_reduce(accsum[:], acc[:], mybir.AxisListType.X, ALU.add)
        nc.sync.dma_start(out_d[:], accsum[:])

    nc.compile()
    return nc


def _get_nc():
    if "nc" not in _NC_CACHE:
        _NC_CACHE["nc"] = build_nc(use_gp=bool(int(
            os.environ.get("COLOR_GP", "1"))))
    return _NC_CACHE["nc"]


def kernel(x: np.ndarray, y: np.ndarray) -> np.ndarray:
    assert x.shape == (32, 3, 512, 512) and y.shape == (32, 3, 512, 512)
    nc = _get_nc()
    shp = (IMGS_PER_CORE, 3, ROWS_PER_IMG, NCHUNK, S)
    xs = np.ascontiguousarray(x, dtype=np.float32)
    ys = np.ascontiguousarray(y, dtype=np.float32)
    in_maps = []
    for c in range(NCORE):
        xi = xs[c * IMGS_PER_CORE:(c + 1) * IMGS_PER_CORE].reshape(shp)
        yi = ys[c * IMGS_PER_CORE:(c + 1) * IMGS_PER_CORE].reshape(shp)
        in_maps.append({"x": xi, "y": yi})
    trace = bool(int(os.environ.get("COLOR_TRACE", "0")))
    res = run_bass_kernel_spmd(nc, in_maps, core_ids=list(range(NCORE)),
                               trace=trace)
    _NC_CACHE["last_results"] = res
    total = np.float64(0.0)
    for c in range(NCORE):
        total += np.float64(res.results[c]["out"].sum())
    npix = 32 * 512 * 512
    return np.float32(total / npix / 100.0)


# revision 35
# speedup vs baseline: 1.0409x; 1.0183x over previous
"""CIEDE2000 ColorLoss kernel v2.1 for Trainium2, 8 NeuronCores, data-parallel.

Full inputs x, y: [32, 3, 512, 512] f32 NCHW in [0, 1).
Output: scalar f32 = mean(ciede2000(rgb2lab(x), rgb2lab(y))) / 100.

v2 design (vs the v1 baseline):
  - ONE ACT table set (natural_log_exp): all roots/powers as Ln/Exp pairs,
    no Arctan/Sin tables -> no ACT_TABLE_LOAD thrash.
  - No GPSIMD tensor_scalar (19.7us each on HW); GPSIMD only runs plain
    tensor_tensor add/sub/mult offloads.
  - Hue handled vectorially (no angles): (cos h, sin h) unit vectors, the
    CIEDE2000 hbar wrap rule == bisector of the smaller arc, so
    (cos hbar, sin hbar) = normalize(u1+u2). T expands exactly as
    P4(cos hbar) + sin hbar * Q3(cos hbar).
  - dHp via dHp^2 = 2*(c1'c2' - 25a1'a2' - 4b1b2), sign from the cross
    product; dtheta gaussian via cos(hbar-275deg) small-angle identity.
  - sRGB linearization: single fused deg-3 poly (custom DVE op), dark-branch
    dropped (validated: end-to-end rel err ~1e-4).
  - G / RC chroma sigmoids: s^7 power chain + approx reciprocal + deg-3 poly
    of sqrt(1-z), all on DVE.
  - bf16 storage for most intermediates (DVE 2x modes); fp32 where bit
    tricks (reciprocal seed) or accuracy demand it.
  - 12 new fused custom DVE ops registered at import (runtime extension of
    concourse.dve_ops.OPS).
"""
import os
import re
import sys

sys.path.insert(0, "/opt/trn_rl_repo")

import numpy as np
import concourse.bacc as bacc
import concourse.tile as tile
import concourse.mybir as mybir
from concourse.bass_utils import run_bass_kernel_spmd
from contextlib import ExitStack

F32 = mybir.dt.float32
BF16 = mybir.dt.bfloat16
AF = mybir.ActivationFunctionType
ALU = mybir.AluOpType

P = 128
S = 1024          # chunk free dim (pixel pairs per partition-col block)
D = 2 * S         # both-images width
M = 3 * D         # 3 channels / comps width
NCHUNK = 8
NCORE = 8
IMGS_PER_CORE = 4
ROWS_PER_IMG = 32

# ---------------- constants ----------------
_M = np.array([[0.412453, 0.357580, 0.180423],
               [0.212671, 0.715160, 0.072169],
               [0.019334, 0.119193, 0.950227]], dtype=np.float64)
_W = np.array([0.95047, 1.0, 1.08883], dtype=np.float64)
MW = (_M / _W[:, None])            # [3,3] row k = xyz_k coeffs

_d = np.deg2rad
COS30, SIN30 = np.cos(_d(30)), np.sin(_d(30))
COS6, SIN6 = np.cos(_d(6)), np.sin(_d(6))
COS63, SIN63 = np.cos(_d(63)), np.sin(_d(63))
COS275, SIN275 = np.cos(_d(275)), np.sin(_d(275))

# T = P4(cb) + sb*Q3(cb)  (exact 4-harmonic expansion)
E4 = -1.6 * COS63
E3 = 1.28 * COS6
E2 = 0.48 + 1.6 * COS63
E1 = -0.17 * COS30 - 0.96 * COS6
E0 = 1.0 - 0.24 - 0.20 * COS63
Q3_ = [-1.6 * SIN63, -1.28 * SIN6, 0.8 * SIN63, 0.32 * SIN6 - 0.085]
# same polys in (cb/5, sb/2) coordinates (normalize is scale-invariant)
E4S, E3S, E2S, E1S = 625 * E4, 125 * E3, 25 * E2, 5 * E1
Q3S = [250 * Q3_[0], 50 * Q3_[1], 10 * Q3_[2], 2 * Q3_[3]]

K1 = 1.0471976          # 60deg in rad (2*dtheta = K1 * e)
K3 = -K1 ** 3 / 6.0
K5 = K1 ** 5 / 120.0
KG = 2.0 * (180.0 / np.pi) ** 2 / 625.0   # 10.50499: (hbar-275)^2/25^2 ~ KG*(1-cos)

PWA, PWB = 0.9398, 0.3893   # 1-seg PWL hypot coefficients


def _fit_poly(f, lo, hi, deg, w=None, n=20001):
    x = np.linspace(lo, hi, n)
    yv = f(x)
    Wt = np.ones_like(x) if w is None else w(x)
    V = np.vander(x, deg + 1)
    c, *_ = np.linalg.lstsq(V * Wt[:, None], yv * Wt, rcond=None)
    return c            # highest power first


LINC = _fit_poly(lambda x: ((x + 0.055) / 1.055) ** 2.4, 0.0, 1.0, 3)
SQ1Z = _fit_poly(lambda z: np.sqrt(1.0 - z), 0.0, 1.0, 3,
                 w=lambda z: 1.0 / np.sqrt(0.005 + z))
GC = -0.5 * SQ1Z                   # g1 = 1.5 - 0.5*p(z)
GC[-1] += 1.5

_NC_CACHE = {}

# Force every ACT op onto the combined ln+exp table set: hide all other
# sets from the table-load chooser (indices preserved; set 6 has ln, exp,
# square, sign, relu, abs).
import concourse.bacc as _bacc_mod
from concourse.hw_specs import get_activation_tables as _gat_orig

def _gat_one_set(arch):
    tabs = _gat_orig(arch)
    keep = "natural_log_exp_and_others"
    return {n: (f if n == keep else set()) for n, f in tabs.items()}

_bacc_mod.get_activation_tables = _gat_one_set

# ---------------- custom DVE ops ----------------
_OPS = {}


def _register_custom_ops():
    if _OPS:
        return _OPS
    import concourse.dve_ops as _dm
    prior = {o.name: o for o in _dm.OPS if o.name.startswith("ANT_")}
    if prior:            # already registered in this process (re-import)
        _OPS.update(prior)
        return _OPS
    from concourse.dve_spec import (
        Spec, Src0, Src1, C0, C1, C2, C3, Zero, One, relu, maxx, minn, lower,
        _spill_c3_to_src1,
    )
    import concourse.dve_ops as dmod

    def mk(name, body, ref, spill=False):
        spec = Spec(body=_spill_c3_to_src1(body) if spill else body,
                    reference=ref)
        row = max(dmod._SUB_OPCODE_FOR_NAME.values()) + 1
        assert row < 0x20, "custom DVE opcode rows exhausted"
        dmod._SUB_OPCODE_FOR_NAME[name] = row
        op = dmod.DveOp(name, spec, subdim=False, uops_sha={})
        for ver in ("v3", "v4"):
            try:
                op.compile(ver)
            except ValueError as e:
                m = re.search(r"([0-9a-f]{16})\s*≠\s*pinned", str(e))
                if not m:
                    raise
                op.uops_sha[ver] = m.group(1)
                op.compile(ver)
            except Exception:
                pass        # ver not supported; fine if unused
        dmod.OPS.append(op)
        dmod.CUSTOM_DVE_SPECS[name] = op.spec
        _OPS[name] = op
        return op

    # deg-3 Horner; d0 spilled to Src1 ([P,1] broadcast tile)
    mk("ANT_POLY3", ((C0 * Src0 + C1) * Src0 + C2) * Src0 + C3,
       lambda in0, in1, s0, s1, imm2:
           (((s0 * in0.astype(np.float32) + s1) * in0 + imm2) * in0
            + in1).astype(np.float32), spill=True)
    # deg-2 Horner
    mk("ANT_POLY2", (C0 * Src0 + C1) * Src0 + C2,
       lambda in0, in1, s0, s1, imm2:
           ((s0 * in0.astype(np.float32) + s1) * in0 + imm2).astype(np.float32))
    # p = (h*c + C0)*c + C1   (tail of deg-4 Horner; in0=c, in1=h)
    mk("ANT_TP2B", (Src1 * Src0 + C0) * Src0 + C1,
       lambda in0, in1, s0, s1, imm2:
           ((in1.astype(np.float32) * in0 + s0) * in0 + s1).astype(np.float32))
    # (C0*a)^2 + (C1*b)^2
    _qa = (Src0 * C0) * (Src0 * C0)
    _qb = (Src1 * C1) * (Src1 * C1)
    mk("ANT_Q2", _qa + _qb,
       lambda in0, in1, s0, s1, imm2:
           ((s0 * in0.astype(np.float32)) ** 2 + (s1 * in1) ** 2)
           .astype(np.float32))
    # C0 * (a+b)^7 + C1  (fused chroma-sum sigmoid input)
    _ss = Src0 + Src1
    _t2 = _ss * _ss
    _t4 = _t2 * _t2
    mk("ANT_S7C2", (_t4 * _t2 * _ss) * C0 + C1,
       lambda in0, in1, s0, s1, imm2:
           (s0 * (in0.astype(np.float64) + in1) ** 7 + s1).astype(np.float32))
    # (a*b)^2
    _ab = Src0 * Src1
    mk("ANT_TSQ", _ab * _ab,
       lambda in0, in1, s0, s1, imm2:
           ((in0.astype(np.float32) * in1) ** 2).astype(np.float32))
    # C0 * s^7 + C1
    _s2 = Src0 * Src0
    _s4 = _s2 * _s2
    _s7 = _s4 * _s2 * Src0
    mk("ANT_S7C", _s7 * C0 + C1,
       lambda in0, in1, s0, s1, imm2:
           (s0 * in0.astype(np.float64) ** 7 + s1).astype(np.float32))
    # C0*a + C1*b
    mk("ANT_DOT2", Src0 * C0 + Src1 * C1,
       lambda in0, in1, s0, s1, imm2:
           (s0 * in0.astype(np.float32) + s1 * in1).astype(np.float32))
    # e * (K1 + K3 e^2 + K5 e^4): sin(K1*e) odd poly
    _e2 = Src0 * Src0
    mk("ANT_SINP", ((C0 * _e2 + C1) * _e2 + C2) * Src0,
       lambda in0, in1, s0, s1, imm2:
           (((s0 * in0.astype(np.float32) ** 2 + s1) * in0 ** 2 + imm2) * in0)
           .astype(np.float32))
    # a^2 + b^2
    mk("ANT_FQ1", Src0 * Src0 + Src1 * Src1,
       lambda in0, in1, s0, s1, imm2:
           (in0.astype(np.float32) ** 2 + in1.astype(np.float32) ** 2)
           .astype(np.float32))
    # a^2 + b
    mk("ANT_FQ2", Src0 * Src0 + Src1,
       lambda in0, in1, s0, s1, imm2:
           (in0.astype(np.float32) ** 2 + in1).astype(np.float32))
    # relu(a + b)
    mk("ANT_FSR", relu(Src0 + Src1),
       lambda in0, in1, s0, s1, imm2:
           np.maximum(in0.astype(np.float32) + in1, 0.0).astype(np.float32))
    # ((a*C0)*b + 1) * C1   (S-scale with folded output scale)
    mk("ANT_AFFS", ((Src0 * C0) * Src1 + One) * C1,
       lambda in0, in1, s0, s1, imm2:
           ((1.0 + s0 * in0.astype(np.float32) * in1) * s1).astype(np.float32))
    return _OPS


_BIASES = [0.055 / 1.055, -66.0, 20.0, -float(KG), 1.0, float(np.log(2.0)),
           float(np.log(20.0))]


def _reg_consts(nc, values):
    for v in values:
        v = float(v)
        if (F32, v) not in nc.const_aps.aps:
            t = nc.alloc_sbuf_tensor(f"constf32_{repr(v)}", [128, 1], F32)
            nc.gpsimd.memset(t.ap(), v)
            nc.const_aps.aps[(F32, v)] = t.ap()
    nc.all_engine_barrier()


# tt ops that may be offloaded to GPSIMD (plain add/sub/mult only)
GP_OPS = {"m1", "m2", "m3", "v1", "v2", "tu", "Tt", "Wa", "Wb", "sfy",
          "dfy", "dCp", "cbb", "sbb", "rr"}


def build_nc(use_gp=True):
    ops = _register_custom_ops()
    POLY3, POLY2, TP2B = ops["ANT_POLY3"], ops["ANT_POLY2"], ops["ANT_TP2B"]
    Q2, S7C = ops["ANT_Q2"], ops["ANT_S7C"]
    DOT2, SINP = ops["ANT_DOT2"], ops["ANT_SINP"]
    FQ1, FQ2, FSR = ops["ANT_FQ1"], ops["ANT_FQ2"], ops["ANT_FSR"]
    AFFS = ops["ANT_AFFS"]
    S7C2, TSQ = ops["ANT_S7C2"], ops["ANT_TSQ"]

    nc = bacc.Bacc("TRN2", target_bir_lowering=False, debug=False)
    _reg_consts(nc, _BIASES)
    A = nc.scalar
    V = nc.vector
    Gp = nc.gpsimd

    shp = [IMGS_PER_CORE, 3, ROWS_PER_IMG, NCHUNK, S]
    x_d = nc.dram_tensor("x", shp, F32, kind="ExternalInput").ap()
    y_d = nc.dram_tensor("y", shp, F32, kind="ExternalInput").ap()
    out_d = nc.dram_tensor("out", [P, 1], F32, kind="ExternalOutput").ap()

    with tile.TileContext(nc) as tc, ExitStack() as ctx:
        pool = ctx.enter_context(tc.tile_pool(name="main", bufs=1))

        def TL(tag, w, dt):
            return pool.tile([P, w], dt, tag=tag, name=tag)

        # broadcast-constant [P,1] tiles (POLY3 d0 via C3-spill)
        def bctile(tag, val):
            t = pool.tile([P, 1], F32, tag=tag, name=tag)
            nc.gpsimd.memset(t[:], float(val))
            return t

        bc_lin = bctile("bc_lin", LINC[3])
        bc_g = bctile("bc_g", GC[3])
        bc_rc = bctile("bc_rc", SQ1Z[3])
        bc_tq = bctile("bc_tq", Q3S[3])
        nc.all_engine_barrier()

        acc = pool.tile([P, NCHUNK], F32, tag="acc", name="acc")

        NTMPF = 3      # rotating [P,S] f32 scratch
        NTMPB = 7      # rotating [P,S] bf16 scratch
        tmpi = [0, 0]

        def tmpf():
            t = TL(f"tmpf{tmpi[0] % NTMPF}", S, F32)
            tmpi[0] += 1
            return t

        def tmpb():
            t = TL(f"tmpb{tmpi[1] % NTMPB}", S, BF16)
            tmpi[1] += 1
            return t

        def emit_tt(name, out, a, b, op):
            if use_gp and name in GP_OPS:
                Gp.tensor_tensor(out[:], a, b, op)
            else:
                V.tensor_tensor(out[:], a, b, op)

        for k in range(NCHUNK):
            # ---- DMA input: rgb mega tile [P, 6144] = 3ch x (x|y) ----
            # alternating tag: chunk k+1 DMA overlaps chunk k compute
            rgb = TL(f"mega_rgb{k % 2}", M, F32)
            for c in range(3):
                for im in range(IMGS_PER_CORE):
                    pr = slice(im * ROWS_PER_IMG, (im + 1) * ROWS_PER_IMG)
                    nc.sync.dma_start(rgb[pr, c * D:c * D + S],
                                      x_d[im, c, :, k, :])
                    nc.sync.dma_start(rgb[pr, c * D + S:(c + 1) * D],
                                      y_d[im, c, :, k, :])

            # ---- linearize: deg-3 poly per channel (finer overlap) ----
            lin = TL("mega_lin", M, BF16)
            for c in range(3):
                V._custom_dve(POLY3, out=lin[:, c * D:(c + 1) * D],
                              in0=rgb[:, c * D:(c + 1) * D], in1=bc_lin[:],
                              s0=float(LINC[0]), s1=float(LINC[1]),
                              imm2=float(LINC[2]))

            # ---- xyz mix (per comp: DOT2 + stt) ----
            X = TL("mega_x", M, BF16)
            for kk in range(3):
                lr = lin[:, 0:D]
                lg = lin[:, D:2 * D]
                lb = lin[:, 2 * D:3 * D]
                t0 = TL("dT0", D, BF16)
                V._custom_dve(DOT2, out=t0[:], in0=lr, in1=lg,
                              s0=float(MW[kk, 0]), s1=float(MW[kk, 1]))
                V.scalar_tensor_tensor(X[:, kk * D:(kk + 1) * D], lb,
                                       float(MW[kk, 2]), t0[:],
                                       ALU.mult, ALU.add)

            # ---- cbrt via Ln/Exp per comp (one table set) ----
            LX = TL(f"mega_rgb{k % 2}", M, F32)  # rgb dead after lin poly
            f = TL("mega_f", M, BF16)
            for c in range(3):
                sl = slice(c * D, (c + 1) * D)
                A.activation(LX[:, sl], X[:, sl], AF.Ln)
                A.activation(f[:, sl], LX[:, sl], AF.Exp,
                             scale=float(1.0 / 3.0))

            fx = f[:, 0:D]
            fy = f[:, D:2 * D]
            fz = f[:, 2 * D:3 * D]
            fy1 = f[:, D:D + S]
            fy2 = f[:, D + S:2 * D]

            alpha = TL("alpha", D, BF16)
            emit_tt("alpha", alpha, fx, fy, ALU.subtract)
            beta = TL("beta", D, BF16)
            emit_tt("beta", beta, fy, fz, ALU.subtract)
            be1 = beta[:, 0:S]
            be2 = beta[:, S:D]

            # ---- pre-G chroma (exact, ACT sqrt) + G sigmoid ----
            qpre = TL("dP", D, F32)
            V._custom_dve(Q2, out=qpre[:], in0=alpha[:], in1=beta[:],
                          s0=5.0, s1=2.0)
            A.activation(qpre[:], qpre[:], AF.Ln)
            cpw = TL("dW", D, BF16)
            A.activation(cpw[:], qpre[:], AF.Exp, scale=0.5)
            z7 = tmpf()
            V._custom_dve(S7C2, out=z7[:], in0=cpw[:, 0:S], in1=cpw[:, S:D],
                          s0=128.0, s1=1.0)
            zr = tmpf()
            V.reciprocal_approx_fast(zr[:], z7[:])
            lz = tmpf()
            A.activation(lz[:], zr[:], AF.Ln, scale=-1.0, bias=1.0)
            wg = tmpb()
            A.activation(wg[:], lz[:], AF.Exp, scale=0.5)
            g1 = TL("g1", S, BF16)
            V.tensor_scalar(g1[:], wg[:], -0.5, 1.5, ALU.mult, ALU.add)

            # ---- post-G chroma ----
            ap = TL("ap", D, BF16)
            V.tensor_tensor(ap[:, 0:S], alpha[:, 0:S], g1[:], ALU.mult)
            V.tensor_tensor(ap[:, S:D], alpha[:, S:D], g1[:], ALU.mult)
            qp = TL("dQ", D, F32)
            V._custom_dve(Q2, out=qp[:], in0=ap[:], in1=beta[:],
                          s0=5.0, s1=2.0)
            A.activation(qp[:], qp[:], AF.Ln)
            yq = TL("yq", D, BF16)
            A.activation(yq[:], qp[:], AF.Exp, scale=-0.5)
            cp = TL("cp", D, BF16)
            A.activation(cp[:], qp[:], AF.Exp, scale=0.5)

            dCp = TL("dCp", S, BF16)
            emit_tt("dCp", dCp, cp[:, S:D], cp[:, 0:S], ALU.subtract)
            Scp = TL("Scp", S, F32)
            V.tensor_tensor(Scp[:], cp[:, 0:S], cp[:, S:D], ALU.add)
            SCf = tmpf()
            V.tensor_scalar(SCf[:], Scp[:], 0.0225, 0.01, ALU.mult, ALU.add)
            lsc = tmpf()
            A.activation(lsc[:], SCf[:], AF.Ln)
            iC = TL("iC", S, F32)
            A.activation(iC[:], lsc[:], AF.Exp, scale=-1.0)

            # ---- RC sigmoid (same shape as G) ----
            z7c = tmpf()
            V._custom_dve(S7C, out=z7c[:], in0=Scp[:], s0=128.0, s1=1.0)
            zrc = tmpf()
            V.reciprocal_approx_fast(zrc[:], z7c[:])
            lzc = tmpf()
            A.activation(lzc[:], zrc[:], AF.Ln, scale=-1.0, bias=1.0)
            rsqC = TL("rsqC", S, BF16)
            A.activation(rsqC[:], lzc[:], AF.Exp, scale=0.5)

            # ---- SL ----
            sfy = TL("sfy", S, BF16)
            emit_tt("sfy", sfy, fy1, fy2, ALU.add)
            dfy = TL("dfy", S, BF16)
            emit_tt("dfy", dfy, fy2, fy1, ALU.subtract)
            L50 = TL("L50", S, F32)
            A.activation(L50[:], sfy[:], AF.Square, scale=58.0, bias=-66.0)
            l20 = tmpf()
            A.activation(l20[:], L50[:], AF.Ln, bias=20.0)
            rsq20 = tmpb()
            A.activation(rsq20[:], l20[:], AF.Exp, scale=-0.5)
            SLf = tmpf()
            V._custom_dve(AFFS, out=SLf[:], in0=L50[:], in1=rsq20[:],
                          s0=0.015, s1=float(1.0 / 116.0))
            iL = TL("iL", S, F32)
            V.reciprocal_approx_fast(iL[:], SLf[:])

            # ---- unit chroma vectors + bisector ----
            ca = TL("dA", D, BF16)
            V.tensor_tensor(ca[:], ap[:], yq[:], ALU.mult)
            cbt = TL("dB", D, BF16)
            V.tensor_tensor(cbt[:], beta[:], yq[:], ALU.mult)
            Wa = TL("Wa", S, BF16)
            emit_tt("Wa", Wa, ca[:, 0:S], ca[:, S:D], ALU.add)
            Wb = TL("Wb", S, BF16)
            emit_tt("Wb", Wb, cbt[:, 0:S], cbt[:, S:D], ALU.add)
            n2 = tmpf()
            V._custom_dve(Q2, out=n2[:], in0=Wa[:], in1=Wb[:], s0=5.0, s1=2.0)
            ln2 = tmpf()
            A.activation(ln2[:], n2[:], AF.Ln)
            rn = tmpb()
            A.activation(rn[:], ln2[:], AF.Exp, scale=-0.5)
            cbb = TL("cbb", S, BF16)
            emit_tt("cbb", cbb, Wa[:], rn[:], ALU.mult)
            sbb = TL("sbb", S, BF16)
            emit_tt("sbb", sbb, Wb[:], rn[:], ALU.mult)

            # ---- T = P4(cb) + sb*Q3(cb) ----
            q3t = tmpb()
            V._custom_dve(POLY3, out=q3t[:], in0=cbb[:], in1=bc_tq[:],
                          s0=float(Q3S[0]), s1=float(Q3S[1]),
                          imm2=float(Q3S[2]))
            hh = tmpb()
            V._custom_dve(POLY2, out=hh[:], in0=cbb[:],
                          s0=float(E4S), s1=float(E3S), imm2=float(E2S))
            pp = tmpb()
            V._custom_dve(TP2B, out=pp[:], in0=cbb[:], in1=hh[:],
                          s0=float(E1S), s1=float(E0))
            tu = tmpb()
            emit_tt("tu", tu, sbb[:], q3t[:], ALU.mult)
            Tt = tmpb()
            emit_tt("Tt", Tt, pp[:], tu[:], ALU.add)
            SHf = tmpf()
            V._custom_dve(AFFS, out=SHf[:], in0=Scp[:], in1=Tt[:],
                          s0=0.75, s1=0.01)
            lsh = tmpf()
            A.activation(lsh[:], SHf[:], AF.Ln)
            iH = TL("iH", S, F32)
            A.activation(iH[:], lsh[:], AF.Exp, scale=-1.0)

            # ---- RT: gaussian via cos identity + sin poly ----
            cd = tmpb()
            V._custom_dve(DOT2, out=cd[:], in0=cbb[:], in1=sbb[:],
                          s0=float(5 * COS275), s1=float(2 * SIN275))
            ee = tmpb()
            A.activation(ee[:], cd[:], AF.Exp, scale=float(KG),
                         bias=-float(KG))
            s2d = tmpb()
            V._custom_dve(SINP, out=s2d[:], in0=ee[:],
                          s0=float(K5), s1=float(K3), imm2=float(K1))
            rr = TL("rr", S, BF16)
            emit_tt("rr", rr, s2d[:], rsqC[:], ALU.mult)

            # ---- signed dHp = 2*sqrt(c1'c2') * sin(dh/2) ----
            # sin(dh/2) = (u1 x u2) * rn exactly (sign automatic)
            lqs = tmpf()
            V.tensor_tensor(lqs[:], qp[:, 0:S], qp[:, S:D], ALU.add)
            sq2 = tmpb()
            A.activation(sq2[:], lqs[:], AF.Exp, scale=-0.25,
                         bias=float(np.log(20.0)))
            v1 = tmpb()
            emit_tt("v1", v1, ap[:, 0:S], be2, ALU.mult)
            v2 = tmpb()
            emit_tt("v2", v2, ap[:, S:D], be1, ALU.mult)
            cr = tmpf()
            V.tensor_tensor(cr[:], v1[:], v2[:], ALU.subtract)
            sdh = tmpb()
            V.tensor_tensor(sdh[:], cr[:], rn[:], ALU.mult)
            hn = tmpb()
            V.tensor_tensor(hn[:], sdh[:], sq2[:], ALU.mult)

            # ---- assemble F ----
            tL2 = tmpb()
            V._custom_dve(TSQ, out=tL2[:], in0=dfy[:], in1=iL[:])
            tC = tmpb()
            emit_tt("tC", tC, dCp[:], iC[:], ALU.mult)
            tH = tmpb()
            emit_tt("tH", tH, hn[:], iH[:], ALU.mult)
            F1 = tmpb()
            V._custom_dve(FQ2, out=F1[:], in0=tC[:], in1=tL2[:])
            F2 = tmpb()
            V._custom_dve(FQ2, out=F2[:], in0=tH[:], in1=F1[:])
            ct = tmpb()
            emit_tt("ct", ct, tC[:], tH[:], ALU.mult)
            ctr = tmpb()
            V.scalar_tensor_tensor(ctr[:], rr[:], -2.0, ct[:], ALU.mult,
                                   ALU.mult)
            FS = tmpf()
            V._custom_dve(FSR, out=FS[:], in0=F2[:], in1=ctr[:])
            lF = tmpf()
            A.activation(lF[:], FS[:], AF.Ln)
            dE = tmpf()
            A.activation(dE[:], lF[:], AF.Exp, scale=0.5,
                         accum_out=acc[:, k:k + 1])

        accsum = pool.tile([P, 1], F32, tag="accsum", name="accsum")
        V.tensor_reduce(accsum[:], acc[:], mybir.AxisListType.X, ALU.add)
        nc.sync.dma_start(out_d[:], accsum[:])

    nc.compile()
    return nc


def _get_nc():
    if "nc" not in _NC_CACHE:
        _NC_CACHE["nc"] = build_nc(use_gp=bool(int(
            os.environ.get("COLOR_GP", "1"))))
    return _NC_CACHE["nc"]


def kernel(x: np.ndarray, y: np.ndarray) -> np.ndarray:
    assert x.shape == (32, 3, 512, 512) and y.shape == (32, 3, 512, 512)
    nc = _get_nc()
    shp = (IMGS_PER_CORE, 3, ROWS_PER_IMG, NCHUNK, S)
    xs = np.ascontiguousarray(x, dtype=np.float32)
    ys = np.ascontiguousarray(y, dtype=np.float32)
    in_maps = []
    for c in range(NCORE):
        xi = xs[c * IMGS_PER_CORE:(c + 1) * IMGS_PER_CORE].reshape(shp)
        yi = ys[c * IMGS_PER_CORE:(c + 1) * IMGS_PER_CORE].reshape(shp)
        in_maps.append({"x": xi, "y": yi})
    trace = bool(int(os.environ.get("COLOR_TRACE", "0")))
    res = run_bass_kernel_spmd(nc, in_maps, core_ids=list(range(NCORE)),
                               trace=trace)
    _NC_CACHE["last_results"] = res
    total = np.float64(0.0)
    for c in range(NCORE):
        total += np.float64(res.results[c]["out"].sum())
    npix = 32 * 512 * 512
    return np.float32(total / npix / 100.0)


# revision 36
# speedup vs baseline: 1.0718x; 1.0296x over previous
"""CIEDE2000 ColorLoss kernel v2.1 for Trainium2, 8 NeuronCores, data-parallel.

Full inputs x, y: [32, 3, 512, 512] f32 NCHW in [0, 1).
Output: scalar f32 = mean(ciede2000(rgb2lab(x), rgb2lab(y))) / 100.

v2 design (vs the v1 baseline):
  - ONE ACT table set (natural_log_exp): all roots/powers as Ln/Exp pairs,
    no Arctan/Sin tables -> no ACT_TABLE_LOAD thrash.
  - No GPSIMD tensor_scalar (19.7us each on HW); GPSIMD only runs plain
    tensor_tensor add/sub/mult offloads.
  - Hue handled vectorially (no angles): (cos h, sin h) unit vectors, the
    CIEDE2000 hbar wrap rule == bisector of the smaller arc, so
    (cos hbar, sin hbar) = normalize(u1+u2). T expands exactly as
    P4(cos hbar) + sin hbar * Q3(cos hbar).
  - dHp via dHp^2 = 2*(c1'c2' - 25a1'a2' - 4b1b2), sign from the cross
    product; dtheta gaussian via cos(hbar-275deg) small-angle identity.
  - sRGB linearization: single fused deg-3 poly (custom DVE op), dark-branch
    dropped (validated: end-to-end rel err ~1e-4).
  - G / RC chroma sigmoids: s^7 power chain + approx reciprocal + deg-3 poly
    of sqrt(1-z), all on DVE.
  - bf16 storage for most intermediates (DVE 2x modes); fp32 where bit
    tricks (reciprocal seed) or accuracy demand it.
  - 12 new fused custom DVE ops registered at import (runtime extension of
    concourse.dve_ops.OPS).
"""
import os
import re
import sys

sys.path.insert(0, "/opt/trn_rl_repo")

import numpy as np
import concourse.bacc as bacc
import concourse.tile as tile
import concourse.mybir as mybir
from concourse.bass_utils import run_bass_kernel_spmd
from contextlib import ExitStack

F32 = mybir.dt.float32
BF16 = mybir.dt.bfloat16
AF = mybir.ActivationFunctionType
ALU = mybir.AluOpType

P = 128
S = 1024          # chunk free dim (pixel pairs per partition-col block)
D = 2 * S         # both-images width
M = 3 * D         # 3 channels / comps width
NCHUNK = 8
NCORE = 8
IMGS_PER_CORE = 4
ROWS_PER_IMG = 32

# ---------------- constants ----------------
_M = np.array([[0.412453, 0.357580, 0.180423],
               [0.212671, 0.715160, 0.072169],
               [0.019334, 0.119193, 0.950227]], dtype=np.float64)
_W = np.array([0.95047, 1.0, 1.08883], dtype=np.float64)
MW = (_M / _W[:, None])            # [3,3] row k = xyz_k coeffs

_d = np.deg2rad
COS30, SIN30 = np.cos(_d(30)), np.sin(_d(30))
COS6, SIN6 = np.cos(_d(6)), np.sin(_d(6))
COS63, SIN63 = np.cos(_d(63)), np.sin(_d(63))
COS275, SIN275 = np.cos(_d(275)), np.sin(_d(275))

# T = P4(cb) + sb*Q3(cb)  (exact 4-harmonic expansion)
E4 = -1.6 * COS63
E3 = 1.28 * COS6
E2 = 0.48 + 1.6 * COS63
E1 = -0.17 * COS30 - 0.96 * COS6
E0 = 1.0 - 0.24 - 0.20 * COS63
Q3_ = [-1.6 * SIN63, -1.28 * SIN6, 0.8 * SIN63, 0.32 * SIN6 - 0.085]
# same polys in (cb/5, sb/2) coordinates (normalize is scale-invariant)
E4S, E3S, E2S, E1S = 625 * E4, 125 * E3, 25 * E2, 5 * E1
Q3S = [250 * Q3_[0], 50 * Q3_[1], 10 * Q3_[2], 2 * Q3_[3]]

K1 = 1.0471976          # 60deg in rad (2*dtheta = K1 * e)
K3 = -K1 ** 3 / 6.0
K5 = K1 ** 5 / 120.0
KG = 2.0 * (180.0 / np.pi) ** 2 / 625.0   # 10.50499: (hbar-275)^2/25^2 ~ KG*(1-cos)

PWA, PWB = 0.9398, 0.3893   # 1-seg PWL hypot coefficients


def _fit_poly(f, lo, hi, deg, w=None, n=20001):
    x = np.linspace(lo, hi, n)
    yv = f(x)
    Wt = np.ones_like(x) if w is None else w(x)
    V = np.vander(x, deg + 1)
    c, *_ = np.linalg.lstsq(V * Wt[:, None], yv * Wt, rcond=None)
    return c            # highest power first


LINC = _fit_poly(lambda x: ((x + 0.055) / 1.055) ** 2.4, 0.0, 1.0, 3)
SQ1Z = _fit_poly(lambda z: np.sqrt(1.0 - z), 0.0, 1.0, 3,
                 w=lambda z: 1.0 / np.sqrt(0.005 + z))
GC = -0.5 * SQ1Z                   # g1 = 1.5 - 0.5*p(z)
GC[-1] += 1.5

_NC_CACHE = {}

# Force every ACT op onto the combined ln+exp table set: hide all other
# sets from the table-load chooser (indices preserved; set 6 has ln, exp,
# square, sign, relu, abs).
import concourse.bacc as _bacc_mod
from concourse.hw_specs import get_activation_tables as _gat_orig

def _gat_one_set(arch):
    tabs = _gat_orig(arch)
    keep = "natural_log_exp_and_others"
    return {n: (f if n == keep else set()) for n, f in tabs.items()}

_bacc_mod.get_activation_tables = _gat_one_set

# ---------------- custom DVE ops ----------------
_OPS = {}


def _register_custom_ops():
    if _OPS:
        return _OPS
    import concourse.dve_ops as _dm
    prior = {o.name: o for o in _dm.OPS if o.name.startswith("ANT_")}
    if prior:            # already registered in this process (re-import)
        _OPS.update(prior)
        return _OPS
    from concourse.dve_spec import (
        Spec, Src0, Src1, C0, C1, C2, C3, Zero, One, relu, maxx, minn, lower,
        _spill_c3_to_src1,
    )
    import concourse.dve_ops as dmod

    def mk(name, body, ref, spill=False):
        spec = Spec(body=_spill_c3_to_src1(body) if spill else body,
                    reference=ref)
        row = max(dmod._SUB_OPCODE_FOR_NAME.values()) + 1
        assert row < 0x20, "custom DVE opcode rows exhausted"
        dmod._SUB_OPCODE_FOR_NAME[name] = row
        op = dmod.DveOp(name, spec, subdim=False, uops_sha={})
        for ver in ("v3", "v4"):
            try:
                op.compile(ver)
            except ValueError as e:
                m = re.search(r"([0-9a-f]{16})\s*≠\s*pinned", str(e))
                if not m:
                    raise
                op.uops_sha[ver] = m.group(1)
                op.compile(ver)
            except Exception:
                pass        # ver not supported; fine if unused
        dmod.OPS.append(op)
        dmod.CUSTOM_DVE_SPECS[name] = op.spec
        _OPS[name] = op
        return op

    # deg-3 Horner; d0 spilled to Src1 ([P,1] broadcast tile)
    mk("ANT_POLY3", ((C0 * Src0 + C1) * Src0 + C2) * Src0 + C3,
       lambda in0, in1, s0, s1, imm2:
           (((s0 * in0.astype(np.float32) + s1) * in0 + imm2) * in0
            + in1).astype(np.float32), spill=True)
    # deg-2 Horner
    mk("ANT_POLY2", (C0 * Src0 + C1) * Src0 + C2,
       lambda in0, in1, s0, s1, imm2:
           ((s0 * in0.astype(np.float32) + s1) * in0 + imm2).astype(np.float32))
    # p = (h*c + C0)*c + C1   (tail of deg-4 Horner; in0=c, in1=h)
    mk("ANT_TP2B", (Src1 * Src0 + C0) * Src0 + C1,
       lambda in0, in1, s0, s1, imm2:
           ((in1.astype(np.float32) * in0 + s0) * in0 + s1).astype(np.float32))
    # (C0*a)^2 + (C1*b)^2
    _qa = (Src0 * C0) * (Src0 * C0)
    _qb = (Src1 * C1) * (Src1 * C1)
    mk("ANT_Q2", _qa + _qb,
       lambda in0, in1, s0, s1, imm2:
           ((s0 * in0.astype(np.float32)) ** 2 + (s1 * in1) ** 2)
           .astype(np.float32))
    # C0 * (a+b)^7 + C1  (fused chroma-sum sigmoid input)
    _ss = Src0 + Src1
    _t2 = _ss * _ss
    _t4 = _t2 * _t2
    mk("ANT_S7C2", (_t4 * _t2 * _ss) * C0 + C1,
       lambda in0, in1, s0, s1, imm2:
           (s0 * (in0.astype(np.float64) + in1) ** 7 + s1).astype(np.float32))
    # (a*b)^2
    _ab = Src0 * Src1
    mk("ANT_TSQ", _ab * _ab,
       lambda in0, in1, s0, s1, imm2:
           ((in0.astype(np.float32) * in1) ** 2).astype(np.float32))
    # C0 * s^7 + C1
    _s2 = Src0 * Src0
    _s4 = _s2 * _s2
    _s7 = _s4 * _s2 * Src0
    mk("ANT_S7C", _s7 * C0 + C1,
       lambda in0, in1, s0, s1, imm2:
           (s0 * in0.astype(np.float64) ** 7 + s1).astype(np.float32))
    # C0*a + C1*b
    mk("ANT_DOT2", Src0 * C0 + Src1 * C1,
       lambda in0, in1, s0, s1, imm2:
           (s0 * in0.astype(np.float32) + s1 * in1).astype(np.float32))
    # e * (K1 + K3 e^2 + K5 e^4): sin(K1*e) odd poly
    _e2 = Src0 * Src0
    mk("ANT_SINP", ((C0 * _e2 + C1) * _e2 + C2) * Src0,
       lambda in0, in1, s0, s1, imm2:
           (((s0 * in0.astype(np.float32) ** 2 + s1) * in0 ** 2 + imm2) * in0)
           .astype(np.float32))
    # a^2 + b^2
    mk("ANT_FQ1", Src0 * Src0 + Src1 * Src1,
       lambda in0, in1, s0, s1, imm2:
           (in0.astype(np.float32) ** 2 + in1.astype(np.float32) ** 2)
           .astype(np.float32))
    # a^2 + b
    mk("ANT_FQ2", Src0 * Src0 + Src1,
       lambda in0, in1, s0, s1, imm2:
           (in0.astype(np.float32) ** 2 + in1).astype(np.float32))
    # relu(a + b)
    mk("ANT_FSR", relu(Src0 + Src1),
       lambda in0, in1, s0, s1, imm2:
           np.maximum(in0.astype(np.float32) + in1, 0.0).astype(np.float32))
    # ((a*C0)*b + 1) * C1   (S-scale with folded output scale)
    mk("ANT_AFFS", ((Src0 * C0) * Src1 + One) * C1,
       lambda in0, in1, s0, s1, imm2:
           ((1.0 + s0 * in0.astype(np.float32) * in1) * s1).astype(np.float32))
    return _OPS


_BIASES = [0.055 / 1.055, -66.0, 20.0, -float(KG), 1.0, float(np.log(2.0)),
           float(np.log(20.0))]


def _reg_consts(nc, values):
    for v in values:
        v = float(v)
        if (F32, v) not in nc.const_aps.aps:
            t = nc.alloc_sbuf_tensor(f"constf32_{repr(v)}", [128, 1], F32)
            nc.gpsimd.memset(t.ap(), v)
            nc.const_aps.aps[(F32, v)] = t.ap()
    nc.all_engine_barrier()


# tt ops that may be offloaded to GPSIMD (plain add/sub/mult only)
GP_OPS = {"m1", "m2", "m3", "v1", "v2", "tu", "Tt", "Wa", "Wb", "sfy",
          "dfy", "dCp", "cbb", "sbb", "rr"}


def build_nc(use_gp=True):
    ops = _register_custom_ops()
    POLY3, POLY2, TP2B = ops["ANT_POLY3"], ops["ANT_POLY2"], ops["ANT_TP2B"]
    Q2, S7C = ops["ANT_Q2"], ops["ANT_S7C"]
    DOT2, SINP = ops["ANT_DOT2"], ops["ANT_SINP"]
    FQ1, FQ2, FSR = ops["ANT_FQ1"], ops["ANT_FQ2"], ops["ANT_FSR"]
    AFFS = ops["ANT_AFFS"]
    S7C2, TSQ = ops["ANT_S7C2"], ops["ANT_TSQ"]

    nc = bacc.Bacc("TRN2", target_bir_lowering=False, debug=False)
    _reg_consts(nc, _BIASES)
    A = nc.scalar
    V = nc.vector
    Gp = nc.gpsimd

    shp = [IMGS_PER_CORE, 3, ROWS_PER_IMG, NCHUNK, S]
    x_d = nc.dram_tensor("x", shp, F32, kind="ExternalInput").ap()
    y_d = nc.dram_tensor("y", shp, F32, kind="ExternalInput").ap()
    out_d = nc.dram_tensor("out", [P, 1], F32, kind="ExternalOutput").ap()

    with tile.TileContext(nc) as tc, ExitStack() as ctx:
        pool = ctx.enter_context(tc.tile_pool(name="main", bufs=1))

        def TL(tag, w, dt):
            return pool.tile([P, w], dt, tag=tag, name=tag)

        # broadcast-constant [P,1] tiles (POLY3 d0 via C3-spill)
        def bctile(tag, val):
            t = pool.tile([P, 1], F32, tag=tag, name=tag)
            nc.gpsimd.memset(t[:], float(val))
            return t

        bc_lin = bctile("bc_lin", LINC[3])
        bc_g = bctile("bc_g", GC[3])
        bc_rc = bctile("bc_rc", SQ1Z[3])
        bc_tq = bctile("bc_tq", Q3S[3])
        nc.all_engine_barrier()

        acc = pool.tile([P, NCHUNK], F32, tag="acc", name="acc")

        NTMPF = 3      # rotating [P,S] f32 scratch
        NTMPB = 7      # rotating [P,S] bf16 scratch
        tmpi = [0, 0]

        def tmpf():
            t = TL(f"tmpf{tmpi[0] % NTMPF}", S, F32)
            tmpi[0] += 1
            return t

        def tmpb():
            t = TL(f"tmpb{tmpi[1] % NTMPB}", S, BF16)
            tmpi[1] += 1
            return t

        def emit_tt(name, out, a, b, op):
            if use_gp and name in GP_OPS:
                Gp.tensor_tensor(out[:], a, b, op)
            else:
                V.tensor_tensor(out[:], a, b, op)

        for k in range(NCHUNK):
            # ---- DMA input: rgb mega tile [P, 6144] = 3ch x (x|y) ----
            # alternating tag: chunk k+1 DMA overlaps chunk k compute
            rgb = TL(f"mega_rgb{k % 2}", M, F32)
            for c in range(3):
                for im in range(IMGS_PER_CORE):
                    pr = slice(im * ROWS_PER_IMG, (im + 1) * ROWS_PER_IMG)
                    nc.sync.dma_start(rgb[pr, c * D:c * D + S],
                                      x_d[im, c, :, k, :])
                    nc.sync.dma_start(rgb[pr, c * D + S:(c + 1) * D],
                                      y_d[im, c, :, k, :])

            # ---- linearize: deg-3 poly per channel (finer overlap) ----
            lin = TL("mega_lin", M, BF16)
            for c in range(3):
                V._custom_dve(POLY3, out=lin[:, c * D:(c + 1) * D],
                              in0=rgb[:, c * D:(c + 1) * D], in1=bc_lin[:],
                              s0=float(LINC[0]), s1=float(LINC[1]),
                              imm2=float(LINC[2]))

            # ---- xyz mix (per comp: DOT2 + stt) ----
            X = TL("mega_x", M, BF16)
            for kk in range(3):
                lr = lin[:, 0:D]
                lg = lin[:, D:2 * D]
                lb = lin[:, 2 * D:3 * D]
                t0 = TL("dT0", D, BF16)
                V._custom_dve(DOT2, out=t0[:], in0=lr, in1=lg,
                              s0=float(MW[kk, 0]), s1=float(MW[kk, 1]))
                V.scalar_tensor_tensor(X[:, kk * D:(kk + 1) * D], lb,
                                       float(MW[kk, 2]), t0[:],
                                       ALU.mult, ALU.add)

            # ---- cbrt via Ln/Exp per comp (one table set) ----
            LX = TL(f"mega_rgb{k % 2}", M, F32)  # rgb dead after lin poly
            f = TL("mega_f", M, BF16)
            for c in range(3):
                sl = slice(c * D, (c + 1) * D)
                A.activation(LX[:, sl], X[:, sl], AF.Ln)
                A.activation(f[:, sl], LX[:, sl], AF.Exp,
                             scale=float(1.0 / 3.0))

            fx = f[:, 0:D]
            fy = f[:, D:2 * D]
            fz = f[:, 2 * D:3 * D]
            fy1 = f[:, D:D + S]
            fy2 = f[:, D + S:2 * D]

            alpha = TL("alpha", D, BF16)
            emit_tt("alpha", alpha, fx, fy, ALU.subtract)
            beta = TL("beta", D, BF16)
            emit_tt("beta", beta, fy, fz, ALU.subtract)
            be1 = beta[:, 0:S]
            be2 = beta[:, S:D]

            # ---- pre-G chroma (exact, ACT sqrt) + G sigmoid ----
            qpre = TL("dP", D, F32)
            V._custom_dve(Q2, out=qpre[:], in0=alpha[:], in1=beta[:],
                          s0=5.0, s1=2.0)
            A.activation(qpre[:], qpre[:], AF.Ln)
            cpw = TL("dW", D, BF16)
            A.activation(cpw[:], qpre[:], AF.Exp, scale=0.5)
            z7 = tmpf()
            V._custom_dve(S7C2, out=z7[:], in0=cpw[:, 0:S], in1=cpw[:, S:D],
                          s0=128.0, s1=1.0)
            zr = tmpf()
            V.reciprocal_approx_fast(zr[:], z7[:])
            g1 = TL("g1", S, BF16)
            V._custom_dve(POLY3, out=g1[:], in0=zr[:], in1=bc_g[:],
                          s0=float(GC[0]), s1=float(GC[1]), imm2=float(GC[2]))

            # ---- post-G chroma ----
            ap = TL("ap", D, BF16)
            V.tensor_tensor(ap[:, 0:S], alpha[:, 0:S], g1[:], ALU.mult)
            V.tensor_tensor(ap[:, S:D], alpha[:, S:D], g1[:], ALU.mult)
            qp = TL("dQ", D, F32)
            V._custom_dve(Q2, out=qp[:], in0=ap[:], in1=beta[:],
                          s0=5.0, s1=2.0)
            A.activation(qp[:], qp[:], AF.Ln)
            yq = TL("yq", D, BF16)
            A.activation(yq[:], qp[:], AF.Exp, scale=-0.5)
            cp = TL("cp", D, BF16)
            A.activation(cp[:], qp[:], AF.Exp, scale=0.5)

            dCp = TL("dCp", S, BF16)
            emit_tt("dCp", dCp, cp[:, S:D], cp[:, 0:S], ALU.subtract)
            Scp = TL("Scp", S, F32)
            V.tensor_tensor(Scp[:], cp[:, 0:S], cp[:, S:D], ALU.add)
            SCf = tmpf()
            V.tensor_scalar(SCf[:], Scp[:], 0.0225, 0.01, ALU.mult, ALU.add)
            lsc = tmpf()
            A.activation(lsc[:], SCf[:], AF.Ln)
            iC = TL("iC", S, F32)
            A.activation(iC[:], lsc[:], AF.Exp, scale=-1.0)

            # ---- RC sigmoid (same shape as G) ----
            z7c = tmpf()
            V._custom_dve(S7C, out=z7c[:], in0=Scp[:], s0=128.0, s1=1.0)
            zrc = tmpf()
            V.reciprocal_approx_fast(zrc[:], z7c[:])
            rsqC = TL("rsqC", S, BF16)
            V._custom_dve(POLY3, out=rsqC[:], in0=zrc[:], in1=bc_rc[:],
                          s0=float(SQ1Z[0]), s1=float(SQ1Z[1]),
                          imm2=float(SQ1Z[2]))

            # ---- SL ----
            sfy = TL("sfy", S, BF16)
            emit_tt("sfy", sfy, fy1, fy2, ALU.add)
            dfy = TL("dfy", S, BF16)
            emit_tt("dfy", dfy, fy2, fy1, ALU.subtract)
            L50 = TL("L50", S, F32)
            A.activation(L50[:], sfy[:], AF.Square, scale=58.0, bias=-66.0)
            l20 = tmpf()
            A.activation(l20[:], L50[:], AF.Ln, bias=20.0)
            rsq20 = tmpb()
            A.activation(rsq20[:], l20[:], AF.Exp, scale=-0.5)
            SLf = tmpf()
            V._custom_dve(AFFS, out=SLf[:], in0=L50[:], in1=rsq20[:],
                          s0=0.015, s1=float(1.0 / 116.0))
            iL = TL("iL", S, F32)
            V.reciprocal_approx_fast(iL[:], SLf[:])

            # ---- unit chroma vectors + bisector ----
            ca = TL("dA", D, BF16)
            V.tensor_tensor(ca[:], ap[:], yq[:], ALU.mult)
            cbt = TL("dB", D, BF16)
            V.tensor_tensor(cbt[:], beta[:], yq[:], ALU.mult)
            Wa = TL("Wa", S, BF16)
            emit_tt("Wa", Wa, ca[:, 0:S], ca[:, S:D], ALU.add)
            Wb = TL("Wb", S, BF16)
            emit_tt("Wb", Wb, cbt[:, 0:S], cbt[:, S:D], ALU.add)
            n2 = tmpf()
            V._custom_dve(Q2, out=n2[:], in0=Wa[:], in1=Wb[:], s0=5.0, s1=2.0)
            ln2 = tmpf()
            A.activation(ln2[:], n2[:], AF.Ln)
            rn = tmpb()
            A.activation(rn[:], ln2[:], AF.Exp, scale=-0.5)
            cbb = TL("cbb", S, BF16)
            emit_tt("cbb", cbb, Wa[:], rn[:], ALU.mult)
            sbb = TL("sbb", S, BF16)
            emit_tt("sbb", sbb, Wb[:], rn[:], ALU.mult)

            # ---- T = P4(cb) + sb*Q3(cb) ----
            q3t = tmpb()
            V._custom_dve(POLY3, out=q3t[:], in0=cbb[:], in1=bc_tq[:],
                          s0=float(Q3S[0]), s1=float(Q3S[1]),
                          imm2=float(Q3S[2]))
            hh = tmpb()
            V._custom_dve(POLY2, out=hh[:], in0=cbb[:],
                          s0=float(E4S), s1=float(E3S), imm2=float(E2S))
            pp = tmpb()
            V._custom_dve(TP2B, out=pp[:], in0=cbb[:], in1=hh[:],
                          s0=float(E1S), s1=float(E0))
            tu = tmpb()
            emit_tt("tu", tu, sbb[:], q3t[:], ALU.mult)
            Tt = tmpb()
            emit_tt("Tt", Tt, pp[:], tu[:], ALU.add)
            SHf = tmpf()
            V._custom_dve(AFFS, out=SHf[:], in0=Scp[:], in1=Tt[:],
                          s0=0.75, s1=0.01)
            lsh = tmpf()
            A.activation(lsh[:], SHf[:], AF.Ln)
            iH = TL("iH", S, F32)
            A.activation(iH[:], lsh[:], AF.Exp, scale=-1.0)

            # ---- RT: gaussian via cos identity + sin poly ----
            cd = tmpb()
            V._custom_dve(DOT2, out=cd[:], in0=cbb[:], in1=sbb[:],
                          s0=float(5 * COS275), s1=float(2 * SIN275))
            ee = tmpb()
            A.activation(ee[:], cd[:], AF.Exp, scale=float(KG),
                         bias=-float(KG))
            s2d = tmpb()
            V._custom_dve(SINP, out=s2d[:], in0=ee[:],
                          s0=float(K5), s1=float(K3), imm2=float(K1))
            rr = TL("rr", S, BF16)
            emit_tt("rr", rr, s2d[:], rsqC[:], ALU.mult)

            # ---- signed dHp = 2*sqrt(c1'c2') * sin(dh/2) ----
            # sin(dh/2) = (u1 x u2) * rn exactly (sign automatic)
            lqs = tmpf()
            V.tensor_tensor(lqs[:], qp[:, 0:S], qp[:, S:D], ALU.add)
            sq2 = tmpb()
            A.activation(sq2[:], lqs[:], AF.Exp, scale=-0.25,
                         bias=float(np.log(20.0)))
            v1 = tmpb()
            emit_tt("v1", v1, ap[:, 0:S], be2, ALU.mult)
            v2 = tmpb()
            emit_tt("v2", v2, ap[:, S:D], be1, ALU.mult)
            cr = tmpf()
            V.tensor_tensor(cr[:], v1[:], v2[:], ALU.subtract)
            sdh = tmpb()
            V.tensor_tensor(sdh[:], cr[:], rn[:], ALU.mult)
            hn = tmpb()
            V.tensor_tensor(hn[:], sdh[:], sq2[:], ALU.mult)

            # ---- assemble F ----
            tL2 = tmpb()
            V._custom_dve(TSQ, out=tL2[:], in0=dfy[:], in1=iL[:])
            tC = tmpb()
            emit_tt("tC", tC, dCp[:], iC[:], ALU.mult)
            tH = tmpb()
            emit_tt("tH", tH, hn[:], iH[:], ALU.mult)
            F1 = tmpb()
            V._custom_dve(FQ2, out=F1[:], in0=tC[:], in1=tL2[:])
            F2 = tmpb()
            V._custom_dve(FQ2, out=F2[:], in0=tH[:], in1=F1[:])
            ct = tmpb()
            emit_tt("ct", ct, tC[:], tH[:], ALU.mult)
            ctr = tmpb()
            V.scalar_tensor_tensor(ctr[:], rr[:], -2.0, ct[:], ALU.mult,
                                   ALU.mult)
            FS = tmpf()
            V._custom_dve(FSR, out=FS[:], in0=F2[:], in1=ctr[:])
            lF = tmpf()
            A.activation(lF[:], FS[:], AF.Ln)
            dE = tmpf()
            A.activation(dE[:], lF[:], AF.Exp, scale=0.5,
                         accum_out=acc[:, k:k + 1])

        accsum = pool.tile([P, 1], F32, tag="accsum", name="accsum")
        V.tensor_reduce(accsum[:], acc[:], mybir.AxisListType.X, ALU.add)
        nc.sync.dma_start(out_d[:], accsum[:])

    nc.compile()
    return nc


def _get_nc():
    if "nc" not in _NC_CACHE:
        _NC_CACHE["nc"] = build_nc(use_gp=bool(int(
            os.environ.get("COLOR_GP", "1"))))
    return _NC_CACHE["nc"]


def kernel(x: np.ndarray, y: np.ndarray) -> np.ndarray:
    assert x.shape == (32, 3, 512, 512) and y.shape == (32, 3, 512, 512)
    nc = _get_nc()
    shp = (IMGS_PER_CORE, 3, ROWS_PER_IMG, NCHUNK, S)
    xs = np.ascontiguousarray(x, dtype=np.float32)
    ys = np.ascontiguousarray(y, dtype=np.float32)
    in_maps = []
    for c in range(NCORE):
        xi = xs[c * IMGS_PER_CORE:(c + 1) * IMGS_PER_CORE].reshape(shp)
        yi = ys[c * IMGS_PER_CORE:(c + 1) * IMGS_PER_CORE].reshape(shp)
        in_maps.append({"x": xi, "y": yi})
    trace = bool(int(os.environ.get("COLOR_TRACE", "0")))
    res = run_bass_kernel_spmd(nc, in_maps, core_ids=list(range(NCORE)),
                               trace=trace)
    _NC_CACHE["last_results"] = res
    total = np.float64(0.0)
    for c in range(NCORE):
        total += np.float64(res.results[c]["out"].sum())
    npix = 32 * 512 * 512
    return np.float32(total / npix / 100.0)
